# revision 45
# baseline (speedup 1.0000x reference)
"""Trainium2 Bass kernel for a dense transformer decoder layer (fp32 I/O).

Model: B=4, T=2048, H=16 heads, DH=64, D=1024, DFF=4096.
  qkv = x @ w_qkv + b_qkv ; non-causal attention (mask==1) ; residual+LN1 ;
  relu FFN (D->DFF->D) ; residual+LN2.

Sharding over 8 NeuronCores: core c handles batch b=c//2 and head-group
hg=c%2 (8 of 16 heads) for QKV+attention over the full sequence; the core
pair (2b, 2b+1) exchanges attention-output halves with a pairwise
AllReduce(add), and each core runs LN1/FFN/LN2 for its 1024-token half.

v3 design notes (vs v2):
  - QKV / FFN1 / FFN2 matmuls run fp8e4 DoubleRow (2 contraction rows per
    PE cell, ~1.4x tensor throughput).  Weights are host-pre-scaled into
    the e4m3 normal range (w_qkv*32, w_ff*32, w_out*64) and activations
    are scaled (x*16, ln1-out*16, h*8); the descale folds into the
    existing bias/residual ops so op count is unchanged.
  - Attention (hp, tb) loop runs partner-token halves (tb=2,3) first and
    emits the pairwise AllReduce in 4 per-head-pair chunks, overlapping
    the collective under the remaining attention compute.
  - FFN2 iterates token halves with w_out resident in SBUF (fp8), so
    LN2+store for half 0 overlaps FFN2 compute for half 1.
  - FFN1 folds relu+bias+descale+fp8-quant into one ACT op
    (relu(psum/64 + 8*bff) == 8*relu(psum/512 + bff)).
  - b_out folds into the LN1 bias of the fp32 path (xn is only consumed
    by the FFN2 residual add).
"""
import os
import sys
import types

import numpy as np
import ml_dtypes

if "/opt/trn_rl_repo" not in sys.path:
    sys.path.insert(0, "/opt/trn_rl_repo")

BF16NP = ml_dtypes.bfloat16
E4NP = ml_dtypes.float8_e4m3

B, T, H, DH = 4, 2048, 16, 64
D = H * DH            # 1024
DFF = 4096
LN_EPS = 1e-5
N_CORES = 8
TLOC = T // 2         # tokens per core in the FFN phase
HLOC = H // 2         # heads per core

A16 = 184.6650        # 2^7 / ln 2 (bf16 bit-space Schraudolph)
B16 = 16250.0

SX = 16.0             # x scale into QKV (fp8)
SW = 32.0             # w_qkv scale
SX1 = 16.0            # LN1-out scale into FFN1
SWF = 32.0            # w_ff scale
SH = 8.0              # h scale into FFN2
SWO = 64.0            # w_out scale

_PROGRAM = None


def _install_ntff_hook():
    try:
        import antenv
        if "antenv.axon_hooks" in sys.modules:
            return
        mod = types.ModuleType("antenv.axon_hooks")
        holder = [None]
        mod.set_axon_ntff_profile_hook = lambda h: holder.__setitem__(0, h)
        mod.get_axon_ntff_profile_hook = lambda: holder[0]
        sys.modules["antenv.axon_hooks"] = mod
        antenv.axon_hooks = mod
        from trn_agent_boot.trn_boot import _ntff_profile_via_ctypes
        mod.set_axon_ntff_profile_hook(
            _ntff_profile_via_ctypes("/opt/axon/libaxon_pjrt.so"))
    except Exception:
        pass


def _build_program():
    import concourse.bass as bass
    import concourse.mybir as mybir
    import concourse.tile as tile
    from concourse import bacc

    F32 = mybir.dt.float32
    BF = mybir.dt.bfloat16
    E4 = mybir.dt.float8e4
    I16 = mybir.dt.int16
    AF = mybir.ActivationFunctionType
    ALU = mybir.AluOpType
    DR = mybir.MatmulPerfMode.DoubleRow

    nc = bacc.Bacc("TRN2", target_bir_lowering=False, debug=False,
                   num_devices=N_CORES)

    xT_d = nc.dram_tensor("xT", [D, TLOC], F32, kind="ExternalInput").ap()
    xb_d = nc.dram_tensor("xb", [D, T], E4, kind="ExternalInput").ap()
    wq_d = nc.dram_tensor("wq", [D, 512], E4, kind="ExternalInput").ap()
    wk_d = nc.dram_tensor("wk", [D, 512], E4, kind="ExternalInput").ap()
    wv_d = nc.dram_tensor("wv", [D, 512], E4, kind="ExternalInput").ap()
    bq_d = nc.dram_tensor("bq", [128, 4], F32, kind="ExternalInput").ap()
    bk_d = nc.dram_tensor("bk", [128, 4], F32, kind="ExternalInput").ap()
    wff_d = nc.dram_tensor("wff", [D, DFF], E4, kind="ExternalInput").ap()
    bff_d = nc.dram_tensor("bff", [128, 32], F32, kind="ExternalInput").ap()
    wout_d = nc.dram_tensor("wout", [DFF, D], E4, kind="ExternalInput").ap()
    lnw1_d = nc.dram_tensor("lnw1", [128, 8], F32, kind="ExternalInput").ap()
    lnb1_d = nc.dram_tensor("lnb1", [128, 8], F32, kind="ExternalInput").ap()
    lnw2_d = nc.dram_tensor("lnw2", [128, 8], F32, kind="ExternalInput").ap()
    lnb2_d = nc.dram_tensor("lnb2", [128, 8], F32, kind="ExternalInput").ap()
    out_d = nc.dram_tensor("outT", [D, TLOC], F32, kind="ExternalOutput").ap()

    with tile.TileContext(nc) as tc:
        constp = tc.alloc_tile_pool(name="const", bufs=1)
        dramp = tc.alloc_tile_pool(name="dram", bufs=1, space="DRAM")

        eps128 = constp.tile([128, 1], F32)
        nc.vector.memset(eps128[:], LN_EPS)
        ln8_128 = constp.tile([128, 1], F32)
        nc.vector.memset(ln8_128[:], 2.0794415416798357)  # ln(8)
        ones_mat = constp.tile([128, 128], BF)
        nc.vector.memset(ones_mat[:].bitcast(mybir.dt.uint16), 0x3F80)
        ones_c64 = constp.tile([1, 64], BF)
        # 16.0: folds the 1/16 V-scale into the denominator broadcast
        nc.vector.memset(ones_c64[:].bitcast(mybir.dt.uint16), 0x4180)

        bias_tiles = {}
        for name, d_ap, w in [("bq", bq_d, 4), ("bk", bk_d, 4),
                              ("bff", bff_d, 32),
                              ("lnw1", lnw1_d, 8), ("lnb1", lnb1_d, 8),
                              ("lnw2", lnw2_d, 8), ("lnb2", lnb2_d, 8)]:
            t = constp.tile([128, w], F32, tag=name)
            nc.sync.dma_start(out=t[:], in_=d_ap)
            bias_tiles[name] = t
        bq_sb, bk_sb = bias_tiles["bq"], bias_tiles["bk"]
        bff_sb = bias_tiles["bff"]
        lnw1_sb, lnb1_sb = bias_tiles["lnw1"], bias_tiles["lnb1"]
        lnw2_sb, lnb2_sb = bias_tiles["lnw2"], bias_tiles["lnb2"]

        # ============ persistent SBUF state ============
        pXM = tc.alloc_tile_pool(name="pXM", bufs=1)
        xm = [pXM.tile([128, TLOC], F32, tag=f"xm{d}", name=f"xm{d}")
              for d in range(8)]
        pXSB = tc.alloc_tile_pool(name="pXSB", bufs=1)
        xsb = [pXSB.tile([128, TLOC], BF, tag=f"xsb{d}", name=f"xsb{d}")
               for d in range(8)]
        # resident fp8 FFN2 weights: pair p covers dff rows (2p,2p+1)*128
        pWO = tc.alloc_tile_pool(name="pWO", bufs=1)
        wout_sb = [pWO.tile([128, 2, D], E4, tag=f"wo{p}", name=f"wo{p}")
                   for p in range(16)]
        pXNB = tc.alloc_tile_pool(name="pXNB", bufs=1)
        xnb = [pXNB.tile([128, 2, TLOC], E4, tag=f"xnb{j}", name=f"xnb{j}")
               for j in range(4)]
        # phase-D pools allocated below the attention pools so LN1 work can
        # overlap late attention without SBUF-reuse anti-dependencies
        ypartp = tc.alloc_tile_pool(name="ypart", bufs=1)
        lnrow = tc.alloc_tile_pool(name="lnrow", bufs=1)
        lnsq = tc.alloc_tile_pool(name="lnsq", bufs=8)
        lntmp = tc.alloc_tile_pool(name="lntmp", bufs=2)
        psD = tc.alloc_tile_pool(name="psD", bufs=1, space="PSUM")
        pY = tc.alloc_tile_pool(name="pY", bufs=1)
        ymine = [pY.tile([128, TLOC], BF, tag=f"ym{i}", name=f"ym{i}")
                 for i in range(4)]
        ysent = [pY.tile([128, TLOC], BF, tag=f"ys{i}", name=f"ys{i}")
                 for i in range(4)]
        pQKV = tc.alloc_tile_pool(name="pQKV", bufs=1)
        qT = [pQKV.tile([128, T], BF, tag=f"qT{i}", name=f"qT{i}")
              for i in range(4)]
        kT = [pQKV.tile([128, T], BF, tag=f"kT{i}", name=f"kT{i}")
              for i in range(4)]
        # fp8 V for DoubleRow PV: s-pair m holds s-chunks (2m, 2m+1);
        # per head 64 dims + ones col (denominator row) + pad to 80
        v_sb = [pQKV.tile([128, 2, 8, 80], E4, tag=f"v{i}", name=f"v{i}")
                for i in range(8)]
        # collective chunks split by partner-token half (bf16 payload)
        cc_in = [dramp.tile([512, 512], BF, tag=f"ccin{i}", name=f"ccin{i}")
                 for i in range(2)]
        cc_out = [dramp.tile([512, 512], BF, tag=f"ccout{i}", name=f"ccout{i}")
                  for i in range(2)]

        # ================= Phase B: QKV projections (fp8 DR) ============
        with tc.tile_pool(name="xbp", bufs=1) as xbp, \
             tc.tile_pool(name="wqk", bufs=1) as wqkp, \
             tc.tile_pool(name="psQK", bufs=5, space="PSUM") as psQK, \
             tc.tile_pool(name="psV", bufs=2, space="PSUM") as psV:
            xb = [xbp.tile([128, 2, T], E4, tag=f"xb{j}", name=f"xb{j}")
                  for j in range(4)]
            wq_sb = [wqkp.tile([128, 2, 512], E4, tag=f"wq{j}", name=f"wq{j}")
                     for j in range(4)]
            wk_sb = [wqkp.tile([128, 2, 512], E4, tag=f"wk{j}", name=f"wk{j}")
                     for j in range(4)]
            wv_sb = [wqkp.tile([128, 2, 512], E4, tag=f"wv{j}", name=f"wv{j}")
                     for j in range(4)]
            for j in range(4):
                for ks in range(2):
                    dd = 2 * j + ks
                    rows = slice(dd * 128, (dd + 1) * 128)
                    nc.sync.dma_start(out=xb[j][:, ks, :], in_=xb_d[rows, :])
                    nc.sync.dma_start(out=wk_sb[j][:, ks, :], in_=wk_d[rows, :])
                    nc.sync.dma_start(out=wq_sb[j][:, ks, :], in_=wq_d[rows, :])
                    nc.sync.dma_start(out=wv_sb[j][:, ks, :], in_=wv_d[rows, :])
            with nc.named_scope("phB_qkv"):
                # q/k projections: stationary w chunk, moving xb
                for ct in range(4):
                    for (w_sb, b_sb, dst) in [(wk_sb, bk_sb, kT),
                                              (wq_sb, bq_sb, qT)]:
                        pss = [psQK.tile([128, 512], F32, tag="qk", name="qk")
                               for _ in range(4)]
                        for j in range(4):
                            for tb in range(4):
                                nc.tensor.matmul(
                                    pss[tb][:],
                                    w_sb[j][:, :, ct * 128:(ct + 1) * 128],
                                    xb[j][:, :, tb * 512:(tb + 1) * 512],
                                    start=(j == 0), stop=(j == 3),
                                    perf_mode=DR)
                        for tb in range(4):
                            nc.vector.tensor_scalar(
                                out=dst[ct][:, tb * 512:(tb + 1) * 512],
                                in0=pss[tb][:],
                                scalar1=1.0 / (SX * SW),
                                scalar2=b_sb[:, ct:ct + 1],
                                op0=ALU.mult, op1=ALU.add)
                # v projection: stationary xb chunk, moving wv
                for tt in range(16):
                    ps = psV.tile([128, 512], F32, tag="v", name="v")
                    for j in range(4):
                        nc.tensor.matmul(
                            ps[:], xb[j][:, :, tt * 128:(tt + 1) * 128],
                            wv_sb[j][:], start=(j == 0), stop=(j == 3),
                            perf_mode=DR)
                    # store 16*v in fp8 (descale 1/512 * 16)
                    nc.vector.tensor_scalar_mul(
                        v_sb[tt // 2][:, tt % 2, :, 0:64],
                        ps[:].rearrange("p (h e) -> p h e", h=8),
                        16.0 / (SX * SW))
                    nc.vector.memset(v_sb[tt // 2][:, tt % 2, :, 64:65], 1.0)

        # prefetch FFN2 weights (resident) while attention runs
        for p in range(16):
            for ks in range(2):
                cc = 2 * p + ks
                nc.sync.dma_start(out=wout_sb[p][:, ks, :],
                                  in_=wout_d[cc * 128:(cc + 1) * 128, :])

        # ================= Phase C: attention =================
        for d in range(8):
            nc.sync.dma_start(out=xm[d][:], in_=xT_d[d * 128:(d + 1) * 128, :])
        with tc.tile_pool(name="psLe", bufs=2, space="PSUM") as psLp, \
             tc.tile_pool(name="psLo", bufs=1, space="PSUM") as psLop, \
             tc.tile_pool(name="psPV", bufs=2, space="PSUM") as psPVp, \
             tc.tile_pool(name="PT", bufs=2) as PTp, \
             tc.tile_pool(name="nrm", bufs=4) as nrmp:
            with nc.named_scope("phC_attn"):
                def normalize(hp, tb, pvs_e, pvs_o):
                    he, ho = 2 * hp, 2 * hp + 1
                    for hh, pvs in ((he, pvs_e), (ho, pvs_o)):
                        dbr = nrmp.tile([1, 512], BF, tag="dbr", name="dbr")
                        nc.scalar.copy(dbr[:], pvs[64:65, :])
                        bcp = psLp.tile([128, 512], F32, tag="Le",
                                        name="bcp")
                        nc.tensor.matmul(bcp[0:64, :], ones_c64[:],
                                         dbr[:], start=True, stop=True)
                        rec = nrmp.tile([64, 512], F32, tag="rec", name="rec")
                        nc.vector.reciprocal_approx_fast(
                            out=rec[:], in_=bcp[0:64, :])
                        row = (hh // 2)
                        half = (hh % 2) * 64
                        ydst = ymine if tb < 2 else ysent
                        ytsl = slice((tb % 2) * 512, (tb % 2) * 512 + 512)
                        yt = ydst[row][half:half + 64, ytsl]
                        nc.vector.tensor_tensor(
                            yt, pvs[0:64, :], rec[:], ALU.mult)
                        if tb >= 2:
                            nc.sync.dma_start(
                                out=cc_in[tb - 2][hh * 64:(hh + 1) * 64, :],
                                in_=yt)

                def emit_cc(tk):
                    nc.gpsimd.collective_compute(
                        "AllReduce", mybir.AluOpType.add,
                        ins=[cc_in[tk][:].opt()],
                        outs=[cc_out[tk][:].opt()],
                        replica_groups=[[0, 1], [2, 3], [4, 5], [6, 7]],
                    )

                # partner-token halves (tb=2 then 3) first, one head pair at
                # a time, so each token-half exchange chunk starts while
                # attention continues on the remaining halves
                ORDER = [(hp, tb) for tb in (2, 3, 0, 1) for hp in range(4)]
                A8 = 11.5415603  # 2^3/ln2 (e4m3 bit-space Schraudolph)
                B8 = 79.6
                LN8 = 2.0794415416798357

                # 18 ACT / 14 DVE chunks per (hp, tb): DVE also runs the
                # softmax normalize, so ACT takes a bit more of the exp
                EXP_PAT = [0, 1] * 16
                EXP_PAT[15] = 0
                EXP_PAT[31] = 0

                def exp_chunk(dst_e4, src_ps, idx):
                    if EXP_PAT[idx % 32] == 0:
                        nc.scalar.activation(dst_e4, src_ps, AF.Exp,
                                             scale=1.0 / 8.0, bias=ln8_128[:])
                    else:
                        nc.vector.tensor_scalar(
                            out=dst_e4.bitcast(mybir.dt.int8), in0=src_ps,
                            scalar1=A8 / 8.0, scalar2=B8,
                            op0=ALU.mult, op1=ALU.add)

                pending = None  # (hp, tb, pvs_e, pvs_o)
                cnt = 0
                for (hp, tb) in ORDER:
                    he, ho = 2 * hp, 2 * hp + 1
                    tsl = slice(tb * 512, (tb + 1) * 512)
                    pvs_e = psPVp.tile([65, 512], F32, tag="pve", name="pve")
                    pvs_o = psPVp.tile([65, 512], F32, tag="pvo", name="pvo")
                    prev = None  # (PT_e, PT_o, sp)
                    for sp in range(8):
                        PT_e = PTp.tile([128, 2, 512], E4, tag="pte",
                                        name="pte")
                        PT_o = PTp.tile([128, 2, 512], mybir.dt.int8,
                                        tag="pto", name="pto")
                        for ks in range(2):
                            s = 2 * sp + ks
                            ssl = slice(s * 128, (s + 1) * 128)
                            psL_e = psLp.tile([128, 512], F32, tag="Le",
                                              name="Le")
                            psL_o = psLop.tile([128, 512], F32, tag="Lo",
                                               name="Lo")
                            nc.tensor.matmul(
                                psL_e[:], kT[hp][0:64, ssl],
                                qT[hp][0:64, tsl],
                                start=True, stop=True, tile_position=(0, 0))
                            nc.tensor.matmul(
                                psL_o[:], kT[hp][64:128, ssl],
                                qT[hp][64:128, tsl],
                                start=True, stop=True, tile_position=(64, 0))
                            # drain previous s-pair's PVs while exp(s) runs
                            if ks == 0 and prev is not None:
                                PT_pe, PT_po, ps_ = prev
                                nc.tensor.matmul(
                                    pvs_e[:], v_sb[ps_][:, :, he, 0:65],
                                    PT_pe[:],
                                    start=(ps_ == 0), stop=(ps_ == 7),
                                    perf_mode=DR)
                                nc.tensor.matmul(
                                    pvs_o[:], v_sb[ps_][:, :, ho, 0:65],
                                    PT_po[:].bitcast(E4),
                                    start=(ps_ == 0), stop=(ps_ == 7),
                                    perf_mode=DR)
                            exp_chunk(PT_e[:, ks, :], psL_e[:], cnt)
                            exp_chunk(PT_o[:, ks, :].bitcast(E4), psL_o[:],
                                      cnt + 1)
                            cnt += 2
                        prev = (PT_e, PT_o, sp)
                    PT_pe, PT_po, ps_ = prev
                    nc.tensor.matmul(
                        pvs_e[:], v_sb[ps_][:, :, he, 0:65],
                        PT_pe[:], start=False, stop=True, perf_mode=DR)
                    nc.tensor.matmul(
                        pvs_o[:], v_sb[ps_][:, :, ho, 0:65],
                        PT_po[:].bitcast(E4),
                        start=False, stop=True, perf_mode=DR)
                    # deferred normalize of the previous iteration; its PE
                    # ops land behind this iteration's dense MM block so the
                    # PE never stalls on the ACT denominator copy
                    if pending is not None:
                        normalize(*pending)
                        # after all 4 head pairs of a partner-token half
                        if pending[1] >= 2 and pending[0] == 3:
                            emit_cc(pending[1] - 2)
                    pending = (hp, tb, pvs_e, pvs_o)
                normalize(*pending)
        pQKV.release()

        # ============ Phase D: exchange + residual + LN1 ============
        def ln_stats_apply(src, srcb, dst, dstb, lnw, lnb, lnws, lnbs,
                           psp, rowp, sqp, tmpp, tbb, fold16,
                           dst_narrow=False, seq_stats=False):
            """One 512-token block of transposed layernorm: stats via bf16
            ones-matmul on srcb, apply to src -> dst (f32) and optionally
            dstb (fp8, 16x-scaled via lnws/lnbs).  dst_narrow: dst tiles
            are [128, 512] (one block) rather than [128, TLOC].
            seq_stats: run sum then sum-of-squares through a single PSUM
            bank (for phases where only one bank is free)."""
            sl = slice(tbb * 512, (tbb + 1) * 512)
            dsl = slice(0, 512) if dst_narrow else sl
            if seq_stats:
                sqs = []
                psum_s = psp.tile([128, 512], F32, tag="lnst", name="lnst")
                for d in range(8):
                    sq = sqp.tile([128, 512], BF, tag="sq", name="sq")
                    (nc.vector if d < 4 else nc.gpsimd).tensor_tensor(
                        sq[:], srcb[d][:, sl], srcb[d][:, sl], ALU.mult)
                    sqs.append(sq)
                    nc.tensor.matmul(psum_s[:], ones_mat[:], srcb[d][:, sl],
                                     start=(d == 0), stop=(d == 7))
                mean = rowp.tile([128, 512], F32, tag="mean", name="mean")
                nc.vector.tensor_scalar_mul(mean[:], psum_s[:], 1.0 / D)
                psum_q = psp.tile([128, 512], F32, tag="lnst", name="lnst")
                for d in range(8):
                    nc.tensor.matmul(psum_q[:], ones_mat[:], sqs[d][:],
                                     start=(d == 0), stop=(d == 7))
            else:
                psum_s = psp.tile([128, 512], F32, tag="lns", name="lns")
                psum_q = psp.tile([128, 512], F32, tag="lnq", name="lnq")
                for d in range(8):
                    sq = sqp.tile([128, 512], BF, tag="sq", name="sq")
                    (nc.vector if d < 4 else nc.gpsimd).tensor_tensor(
                        sq[:], srcb[d][:, sl], srcb[d][:, sl], ALU.mult)
                    nc.tensor.matmul(psum_s[:], ones_mat[:], srcb[d][:, sl],
                                     start=(d == 0), stop=(d == 7))
                    nc.tensor.matmul(psum_q[:], ones_mat[:], sq[:],
                                     start=(d == 0), stop=(d == 7))
                mean = rowp.tile([128, 512], F32, tag="mean", name="mean")
                nc.vector.tensor_scalar_mul(mean[:], psum_s[:], 1.0 / D)
            m2 = rowp.tile([128, 512], F32, tag="m2", name="m2")
            nc.vector.tensor_tensor(m2[:], mean[:], mean[:], ALU.mult)
            var = m2  # in-place: var = psum_q/D - m2
            nc.vector.scalar_tensor_tensor(
                out=var[:], in0=psum_q[:], scalar=1.0 / D, in1=m2[:],
                op0=ALU.mult, op1=ALU.subtract)
            std = rowp.tile([128, 512], F32, tag="std", name="std")
            nc.scalar.activation(std[:], var[:], AF.Sqrt, bias=eps128[:])
            rstd = std  # in-place reciprocal
            nc.vector.reciprocal_approx_fast(out=rstd[:], in_=std[:])
            ms = rowp.tile([128, 512], F32, tag="ms", name="ms")
            nc.vector.tensor_tensor(ms[:], mean[:], rstd[:], ALU.mult)
            for d in range(8):
                # SBUF-only apply ops: offload the tail d-chunks to GPSIMD
                eng = nc.vector if d < 5 else nc.gpsimd
                tmp = tmpp.tile([128, 512], F32, tag="lt", name="lt")
                eng.tensor_tensor(tmp[:], src[d][:, sl], rstd[:],
                                  ALU.mult)
                tmp2 = tmp
                eng.tensor_tensor(tmp2[:], tmp[:], ms[:],
                                  ALU.subtract)
                nc.vector.tensor_scalar(
                    out=dst[d][:, dsl], in0=tmp2[:],
                    scalar1=lnw[:, d:d + 1], scalar2=lnb[:, d:d + 1],
                    op0=ALU.mult, op1=ALU.add)
                if dstb is not None:
                    # fp8 FFN1 input: 16*normalized (ln1 w/b folded into
                    # w_ff/b_ff on the host), plain const scalar -> GPSIMD
                    nc.gpsimd.tensor_scalar_mul(
                        dstb[d // 2][:, d % 2, sl], tmp2[:], 16.0)

        with nc.named_scope("phD_exch_ln1"):
            # per token half: exchange arrives -> residual -> LN1, so
            # half 0 runs while attention still computes my-token halves
            for tbb in range(2):
                tsl = slice(tbb * 512, (tbb + 1) * 512)
                yp = [ypartp.tile([128, 512], BF, tag=f"yp{i}",
                                  name=f"yp{i}") for i in range(4)]
                for r4 in range(4):
                    nc.sync.dma_start(
                        out=yp[r4][:],
                        in_=cc_out[tbb][r4 * 128:(r4 + 1) * 128, :])
                    (nc.vector if r4 < 2 else nc.gpsimd).tensor_tensor(
                        yp[r4][:], yp[r4][:], ysent[r4][:, tsl],
                        ALU.subtract)
                # residual (rotated D order: chunks 0-3 mine, 4-7 partner);
                # v-bias is folded into xT on the host
                for d in range(8):
                    ysrc = (ymine[d][:, tsl] if d < 4
                            else yp[d - 4][:])
                    eng = nc.vector if d < 4 else nc.gpsimd
                    eng.tensor_tensor(
                        xsb[d][:, tsl], ysrc, xm[d][:, tsl], ALU.add)
                    eng.tensor_tensor(
                        xm[d][:, tsl], ysrc, xm[d][:, tsl], ALU.add)
                ln_stats_apply(xm, xsb, xm, xnb, lnw1_sb, lnb1_sb,
                               None, None,
                               psD, lnrow, lnsq, lntmp, tbb, True,
                               seq_stats=True)
        pY.release()

        # ================= Phase E: FFN (fp8 DR) =================
        xn = xm      # LN1 output (f32, + b_out folded) in place
        r2 = xm      # FFN residual written back in place
        with tc.tile_pool(name="wff", bufs=2) as wffp, \
             tc.tile_pool(name="hbuf", bufs=1) as hbufp:
            with nc.named_scope("phE_ffn1"), \
                 tc.tile_pool(name="psH", bufs=4, space="PSUM") as psH:
                h_sb = [hbufp.tile([128, 2, TLOC], E4, tag=f"hb{p}",
                                   name=f"hb{p}") for p in range(16)]
                for blk in range(8):
                    wt = []
                    for j in range(4):
                        w = wffp.tile([128, 2, 512], E4, tag=f"wf{j}",
                                      name=f"wf{j}")
                        for ks in range(2):
                            dd = 2 * j + ks
                            nc.sync.dma_start(
                                out=w[:, ks, :],
                                in_=wff_d[dd * 128:(dd + 1) * 128,
                                          blk * 512:(blk + 1) * 512])
                        wt.append(w)
                    for j2 in range(4):
                        dt_i = blk * 4 + j2
                        for t2 in range(2):
                            sl = slice(t2 * 512, (t2 + 1) * 512)
                            ps = psH.tile([128, 512], F32, tag="h", name="h")
                            for j in range(4):
                                nc.tensor.matmul(
                                    ps[:],
                                    wt[j][:, :, j2 * 128:(j2 + 1) * 128],
                                    xnb[j][:, :, sl],
                                    start=(j == 0), stop=(j == 3),
                                    perf_mode=DR)
                            # h8 = relu(ps/64 + 8*bff) = 8*relu(ps/512+bff)
                            nc.scalar.activation(
                                h_sb[dt_i // 2][:, dt_i % 2, sl], ps[:],
                                AF.Relu,
                                bias=bff_sb[:, dt_i:dt_i + 1],
                                scale=SH / (SX1 * SWF))
            # FFN2 per token half so LN2(half0) overlaps FFN2(half1)
            with tc.tile_pool(name="psO", bufs=3, space="PSUM") as psO, \
                 tc.tile_pool(name="psD2", bufs=2, space="PSUM") as psD2, \
                 tc.tile_pool(name="lnrow2", bufs=1) as lnrow2, \
                 tc.tile_pool(name="lnsq2", bufs=3) as lnsq2, \
                 tc.tile_pool(name="lntmp2", bufs=2) as lntmp2, \
                 tc.tile_pool(name="ost", bufs=1) as ostp:
                with nc.named_scope("phE_ffn2"):
                    for tg in range(2):
                        sl = slice(tg * 512, (tg + 1) * 512)
                        for dd in range(8):
                            pso = psO.tile([128, 512], F32, tag="o", name="o")
                            for p in range(16):
                                nc.tensor.matmul(
                                    pso[:],
                                    wout_sb[p][:, :, dd * 128:(dd + 1) * 128],
                                    h_sb[p][:, :, sl],
                                    start=(p == 0), stop=(p == 15),
                                    perf_mode=DR)
                            # residual: xn already holds ln1out + b_out
                            nc.vector.scalar_tensor_tensor(
                                out=xsb[dd][:, sl], in0=pso[:],
                                scalar=1.0 / (SH * SWO),
                                in1=xn[dd][:, sl], op0=ALU.mult, op1=ALU.add)
                            nc.vector.scalar_tensor_tensor(
                                out=r2[dd][:, sl], in0=pso[:],
                                scalar=1.0 / (SH * SWO),
                                in1=xn[dd][:, sl], op0=ALU.mult, op1=ALU.add)
                        # LN2 + store for this token half
                        o32 = [ostp.tile([128, 512], F32, tag=f"o{d}",
                                         name=f"o{d}") for d in range(8)]
                        ln_stats_apply(
                            r2, xsb, o32,
                            None, lnw2_sb, lnb2_sb, None, None,
                            psD2, lnrow2, lnsq2, lntmp2, tg, False,
                            dst_narrow=True)
                        for d in range(8):
                            nc.sync.dma_start(
                                out=out_d[d * 128:(d + 1) * 128, sl],
                                in_=o32[d][:])
        psD.release()
        lntmp.release()
        lnsq.release()
        lnrow.release()
        ypartp.release()
        pXNB.release()
        pWO.release()
        pXSB.release()
        pXM.release()
        dramp.release()
        constp.release()

    nc.compile()
    return nc


def _get_program():
    global _PROGRAM
    if _PROGRAM is None:
        _PROGRAM = _build_program()
    return _PROGRAM


def _rotations(hg):
    d0 = hg * 512
    drot = (np.arange(D) + d0) % D
    return d0, drot


def _make_in_maps(x, w_qkv, b_qkv, w_ff, b_ff, w_out, b_out,
                  ln1_w, ln1_b, ln2_w, ln2_b):
    # reference packs qkv interleaved: col(h, dh, sel) = h*192 + dh*3 + sel
    hd = np.arange(H * DH)
    qcols = (hd // DH) * (3 * DH) + (hd % DH) * 3
    kcols = qcols + 1
    vcols = qcols + 2
    in_maps = []
    # ln1 folded into the FFN1 weights/bias (FFN1 input is 16*normalized)
    bff_eff = 8.0 * (b_ff + ln1_b @ w_ff)
    for c in range(N_CORES):
        b = c // 2
        hg = c % 2
        t0 = hg * TLOC
        d0, drot = _rotations(hg)
        x_rot = np.concatenate([x[b, t0:t0 + TLOC, :],
                                x[b, TLOC - t0:T - t0, :]], axis=0)[:, drot]
        xT = np.ascontiguousarray(x_rot.T)          # [D, T]
        bv = b_qkv[vcols][drot]
        im = {
            # v-bias pre-folded into the residual input
            "xT": np.ascontiguousarray(xT[:, :TLOC] + bv[:, None]),
            "xb": np.ascontiguousarray((xT * SX).astype(E4NP)),
            "wq": np.ascontiguousarray(
                (w_qkv[drot][:, qcols[d0:d0 + 512]] * SW).astype(E4NP)),
            "wk": np.ascontiguousarray(
                (w_qkv[drot][:, kcols[d0:d0 + 512]] * SW).astype(E4NP)),
            "wv": np.ascontiguousarray(
                (w_qkv[drot][:, vcols[d0:d0 + 512]] * SW).astype(E4NP)),
            "bq": np.ascontiguousarray(
                b_qkv[qcols[d0:d0 + 512]].reshape(4, 128).T),
            "bk": np.ascontiguousarray(
                b_qkv[kcols[d0:d0 + 512]].reshape(4, 128).T),
            "wff": np.ascontiguousarray(
                (w_ff[drot, :] * ln1_w[drot][:, None] * SWF).astype(E4NP)),
            "bff": np.ascontiguousarray(bff_eff.reshape(32, 128).T),
            "wout": np.ascontiguousarray((w_out[:, drot] * SWO).astype(E4NP)),
            "lnw1": np.ascontiguousarray(ln1_w[drot].reshape(8, 128).T),
            "lnb1": np.ascontiguousarray(
                (ln1_b[drot] + b_out[drot]).reshape(8, 128).T),
            "lnw2": np.ascontiguousarray(ln2_w[drot].reshape(8, 128).T),
            "lnb2": np.ascontiguousarray(ln2_b[drot].reshape(8, 128).T),
        }
        in_maps.append(im)
    return in_maps


def _assemble(results):
    out = np.empty((B, T, D), dtype=np.float32)
    for c in range(N_CORES):
        b = c // 2
        hg = c % 2
        _, drot = _rotations(hg)
        inv = np.argsort(drot)
        out[b, hg * TLOC:(hg + 1) * TLOC, :] = results[c]["outT"].T[:, inv]
    return out


def _numpy_fallback(x, mask, w_qkv, b_qkv, w_ff, b_ff, w_out, b_out,
                    ln1_w, ln1_b, ln2_w, ln2_b):
    def ln(v, w, b):
        mu = v.mean(-1, keepdims=True)
        var = ((v - mu) ** 2).mean(-1, keepdims=True)
        return (v - mu) / np.sqrt(var + LN_EPS) * w + b
    b, t, _ = x.shape
    qkv = x @ w_qkv + b_qkv
    qkv = qkv.reshape(b, t, H, DH, 3).transpose(4, 0, 2, 1, 3)
    q, k, v = qkv[0], qkv[1], qkv[2]
    logits = np.einsum("bhtd,bhsd->bhts", q, k) / np.sqrt(DH)
    logits = logits + (1.0 - mask) * -10000.0
    m = logits.max(-1, keepdims=True)
    e = np.exp(logits - m)
    w = e / e.sum(-1, keepdims=True)
    y = np.einsum("bhts,bhsd->bhtd", w, v)
    y = y.transpose(0, 2, 1, 3).reshape(b, t, H * DH)
    x1 = ln(x + y, ln1_w, ln1_b)
    y2 = np.maximum(x1 @ w_ff + b_ff, 0.0) @ w_out + b_out
    return ln(x1 + y2, ln2_w, ln2_b).astype(np.float32)


def kernel(x, mask, w_qkv, b_qkv, w_ff, b_ff, w_out, b_out,
           ln1_w, ln1_b, ln2_w, ln2_b):
    args = [np.ascontiguousarray(np.asarray(a, dtype=np.float32))
            for a in (x, mask, w_qkv, b_qkv, w_ff, b_ff, w_out, b_out,
                      ln1_w, ln1_b, ln2_w, ln2_b)]
    (x, mask, w_qkv, b_qkv, w_ff, b_ff, w_out, b_out,
     ln1_w, ln1_b, ln2_w, ln2_b) = args

    if not np.all(mask == 1.0):
        return _numpy_fallback(x, mask, w_qkv, b_qkv, w_ff, b_ff, w_out, b_out,
                               ln1_w, ln1_b, ln2_w, ln2_b)

    _install_ntff_hook()
    from concourse.bass_utils import run_bass_kernel_spmd

    nc = _get_program()
    in_maps = _make_in_maps(x, w_qkv, b_qkv, w_ff, b_ff, w_out, b_out,
                            ln1_w, ln1_b, ln2_w, ln2_b)

    kw = {}
    if os.environ.get("BASSK_TRACE"):
        kw = dict(trace=True, trace_cores=[0],
                  tmpdir=os.environ.get("BASSK_TRACEDIR", "/tmp/kernel_trace"))
    res = run_bass_kernel_spmd(nc, in_maps, core_ids=list(range(N_CORES)), **kw)
    kernel._last_results = res
    return _assemble(res.results)


# revision 55
# speedup vs baseline: 1.2157x; 1.2157x over previous
"""Trainium2 Bass kernel for a dense transformer decoder layer (fp32 I/O).

Model: B=4, T=2048, H=16 heads, DH=64, D=1024, DFF=4096.
  qkv = x @ w_qkv + b_qkv ; non-causal attention (mask==1) ; residual+LN1 ;
  relu FFN (D->DFF->D) ; residual+LN2.

Sharding over 8 NeuronCores: core c handles batch b=c//2 and head-group
hg=c%2 (8 of 16 heads) for QKV+attention over the full sequence; the core
pair (2b, 2b+1) exchanges attention-output halves with a pairwise
AllReduce(add), and each core runs LN1/FFN/LN2 for its 1024-token half.

v3 design notes (vs v2):
  - QKV / FFN1 / FFN2 matmuls run fp8e4 DoubleRow (2 contraction rows per
    PE cell, ~1.4x tensor throughput).  Weights are host-pre-scaled into
    the e4m3 normal range (w_qkv*32, w_ff*32, w_out*64) and activations
    are scaled (x*16, ln1-out*16, h*8); the descale folds into the
    existing bias/residual ops so op count is unchanged.
  - Attention (hp, tb) loop runs partner-token halves (tb=2,3) first and
    emits the pairwise AllReduce in 4 per-head-pair chunks, overlapping
    the collective under the remaining attention compute.
  - FFN2 iterates token halves with w_out resident in SBUF (fp8), so
    LN2+store for half 0 overlaps FFN2 compute for half 1.
  - FFN1 folds relu+bias+descale+fp8-quant into one ACT op
    (relu(psum/64 + 8*bff) == 8*relu(psum/512 + bff)).
  - b_out folds into the LN1 bias of the fp32 path (xn is only consumed
    by the FFN2 residual add).
"""
import os
import sys
import types

import numpy as np
import ml_dtypes

if "/opt/trn_rl_repo" not in sys.path:
    sys.path.insert(0, "/opt/trn_rl_repo")

BF16NP = ml_dtypes.bfloat16
E4NP = ml_dtypes.float8_e4m3

B, T, H, DH = 4, 2048, 16, 64
D = H * DH            # 1024
DFF = 4096
LN_EPS = 1e-5
N_CORES = 8
TLOC = T // 2         # tokens per core in the FFN phase
HLOC = H // 2         # heads per core

A16 = 184.6650        # 2^7 / ln 2 (bf16 bit-space Schraudolph)
B16 = 16250.0

SX = 16.0             # x scale into QKV (fp8)
SW = 32.0             # w_qkv scale
SX1 = 16.0            # LN1-out scale into FFN1
SWF = 32.0            # w_ff scale
SH = 8.0              # h scale into FFN2
SWO = 64.0            # w_out scale

_PROGRAM = None


def _install_ntff_hook():
    try:
        import antenv
        if "antenv.axon_hooks" in sys.modules:
            return
        mod = types.ModuleType("antenv.axon_hooks")
        holder = [None]
        mod.set_axon_ntff_profile_hook = lambda h: holder.__setitem__(0, h)
        mod.get_axon_ntff_profile_hook = lambda: holder[0]
        sys.modules["antenv.axon_hooks"] = mod
        antenv.axon_hooks = mod
        from trn_agent_boot.trn_boot import _ntff_profile_via_ctypes
        mod.set_axon_ntff_profile_hook(
            _ntff_profile_via_ctypes("/opt/axon/libaxon_pjrt.so"))
    except Exception:
        pass


def _build_program():
    import concourse.bass as bass
    import concourse.mybir as mybir
    import concourse.tile as tile
    from concourse import bacc

    F32 = mybir.dt.float32
    BF = mybir.dt.bfloat16
    E4 = mybir.dt.float8e4
    I16 = mybir.dt.int16
    AF = mybir.ActivationFunctionType
    ALU = mybir.AluOpType
    DR = mybir.MatmulPerfMode.DoubleRow

    nc = bacc.Bacc("TRN2", target_bir_lowering=False, debug=False,
                   num_devices=N_CORES)

    xT_d = nc.dram_tensor("xT", [D, TLOC], F32, kind="ExternalInput").ap()
    xb_d = nc.dram_tensor("xb", [D, T], E4, kind="ExternalInput").ap()
    wq_d = nc.dram_tensor("wq", [D, 512], E4, kind="ExternalInput").ap()
    wk_d = nc.dram_tensor("wk", [D, 512], E4, kind="ExternalInput").ap()
    wv_d = nc.dram_tensor("wv", [D, 512], E4, kind="ExternalInput").ap()
    bq_d = nc.dram_tensor("bq", [128, 4], F32, kind="ExternalInput").ap()
    bk_d = nc.dram_tensor("bk", [128, 4], F32, kind="ExternalInput").ap()
    wff_d = nc.dram_tensor("wff", [D, DFF], E4, kind="ExternalInput").ap()
    bff_d = nc.dram_tensor("bff", [128, 32], F32, kind="ExternalInput").ap()
    wout_d = nc.dram_tensor("wout", [DFF, D], E4, kind="ExternalInput").ap()
    lnw1_d = nc.dram_tensor("lnw1", [128, 8], F32, kind="ExternalInput").ap()
    lnb1_d = nc.dram_tensor("lnb1", [128, 8], F32, kind="ExternalInput").ap()
    lnw2_d = nc.dram_tensor("lnw2", [128, 8], F32, kind="ExternalInput").ap()
    lnb2_d = nc.dram_tensor("lnb2", [128, 8], F32, kind="ExternalInput").ap()
    out_d = nc.dram_tensor("outT", [D, TLOC], F32, kind="ExternalOutput").ap()

    with tile.TileContext(nc) as tc:
        constp = tc.alloc_tile_pool(name="const", bufs=1)
        dramp = tc.alloc_tile_pool(name="dram", bufs=1, space="DRAM")

        eps128 = constp.tile([128, 1], F32)
        nc.vector.memset(eps128[:], LN_EPS)
        ln8_128 = constp.tile([128, 1], F32)
        nc.vector.memset(ln8_128[:], 2.0794415416798357)  # ln(8)
        ones_mat = constp.tile([128, 128], BF)
        nc.vector.memset(ones_mat[:].bitcast(mybir.dt.uint16), 0x3F80)
        ones_c64 = constp.tile([1, 64], BF)
        # 16.0: folds the 1/16 V-scale into the denominator broadcast
        nc.vector.memset(ones_c64[:].bitcast(mybir.dt.uint16), 0x4180)

        bias_tiles = {}
        for name, d_ap, w in [("bq", bq_d, 4), ("bk", bk_d, 4),
                              ("bff", bff_d, 32),
                              ("lnw1", lnw1_d, 8), ("lnb1", lnb1_d, 8),
                              ("lnw2", lnw2_d, 8), ("lnb2", lnb2_d, 8)]:
            t = constp.tile([128, w], F32, tag=name)
            nc.sync.dma_start(out=t[:], in_=d_ap)
            bias_tiles[name] = t
        bq_sb, bk_sb = bias_tiles["bq"], bias_tiles["bk"]
        bff_sb = bias_tiles["bff"]
        lnw1_sb, lnb1_sb = bias_tiles["lnw1"], bias_tiles["lnb1"]
        lnw2_sb, lnb2_sb = bias_tiles["lnw2"], bias_tiles["lnb2"]

        # ============ persistent SBUF state ============
        pXM = tc.alloc_tile_pool(name="pXM", bufs=1)
        xm = [pXM.tile([128, TLOC], F32, tag=f"xm{d}", name=f"xm{d}")
              for d in range(8)]
        pXSB = tc.alloc_tile_pool(name="pXSB", bufs=1)
        xsb = [pXSB.tile([128, TLOC], BF, tag=f"xsb{d}", name=f"xsb{d}")
               for d in range(8)]
        # resident fp8 FFN2 weights: pair p covers dff rows (2p,2p+1)*128
        pWO = tc.alloc_tile_pool(name="pWO", bufs=1)
        wout_sb = [pWO.tile([128, 2, D], E4, tag=f"wo{p}", name=f"wo{p}")
                   for p in range(16)]
        pXNB = tc.alloc_tile_pool(name="pXNB", bufs=1)
        xnb = [pXNB.tile([128, 2, TLOC], E4, tag=f"xnb{j}", name=f"xnb{j}")
               for j in range(4)]
        # phase-D pools allocated below the attention pools so LN1 work can
        # overlap late attention without SBUF-reuse anti-dependencies
        ypartp = tc.alloc_tile_pool(name="ypart", bufs=1)
        lnrow = tc.alloc_tile_pool(name="lnrow", bufs=1)
        lnsq = tc.alloc_tile_pool(name="lnsq", bufs=8)
        lntmp = tc.alloc_tile_pool(name="lntmp", bufs=2)
        # 2 PSUM banks shared by attention logits-even/bcast and LN1 stats
        psLp = tc.alloc_tile_pool(name="psLe", bufs=2, space="PSUM")
        pY = tc.alloc_tile_pool(name="pY", bufs=1)
        ymine = [pY.tile([128, TLOC], BF, tag=f"ym{i}", name=f"ym{i}")
                 for i in range(4)]
        ysent = [pY.tile([128, TLOC], BF, tag=f"ys{i}", name=f"ys{i}")
                 for i in range(4)]
        pQKV = tc.alloc_tile_pool(name="pQKV", bufs=1)
        qT = [pQKV.tile([128, T], BF, tag=f"qT{i}", name=f"qT{i}")
              for i in range(4)]
        kT = [pQKV.tile([128, T], BF, tag=f"kT{i}", name=f"kT{i}")
              for i in range(4)]
        # fp8 V for DoubleRow PV: s-pair m holds s-chunks (2m, 2m+1);
        # per head 64 dims + ones col (denominator row) + pad to 80
        v_sb = [pQKV.tile([128, 2, 8, 80], E4, tag=f"v{i}", name=f"v{i}")
                for i in range(8)]
        # collective chunks split by partner-token half (bf16 payload)
        cc_in = [dramp.tile([512, 512], BF, tag=f"ccin{i}", name=f"ccin{i}")
                 for i in range(2)]
        cc_out = [dramp.tile([512, 512], BF, tag=f"ccout{i}", name=f"ccout{i}")
                  for i in range(2)]

        # ================= Phase B: QKV projections (fp8 DR) ============
        with tc.tile_pool(name="xbp", bufs=1) as xbp, \
             tc.tile_pool(name="wqk", bufs=1) as wqkp, \
             tc.tile_pool(name="psQK", bufs=4, space="PSUM") as psQK, \
             tc.tile_pool(name="psV", bufs=2, space="PSUM") as psV:
            xb = [xbp.tile([128, 2, T], E4, tag=f"xb{j}", name=f"xb{j}")
                  for j in range(4)]
            wq_sb = [wqkp.tile([128, 2, 512], E4, tag=f"wq{j}", name=f"wq{j}")
                     for j in range(4)]
            wk_sb = [wqkp.tile([128, 2, 512], E4, tag=f"wk{j}", name=f"wk{j}")
                     for j in range(4)]
            wv_sb = [wqkp.tile([128, 2, 512], E4, tag=f"wv{j}", name=f"wv{j}")
                     for j in range(4)]
            for j in range(4):
                for ks in range(2):
                    dd = 2 * j + ks
                    rows = slice(dd * 128, (dd + 1) * 128)
                    nc.sync.dma_start(out=xb[j][:, ks, :], in_=xb_d[rows, :])
                    nc.sync.dma_start(out=wk_sb[j][:, ks, :], in_=wk_d[rows, :])
                    nc.sync.dma_start(out=wq_sb[j][:, ks, :], in_=wq_d[rows, :])
                    nc.sync.dma_start(out=wv_sb[j][:, ks, :], in_=wv_d[rows, :])
            with nc.named_scope("phB_qkv"):
                # q/k projections: stationary w chunk, moving xb
                for ct in range(4):
                    for (w_sb, b_sb, dst) in [(wk_sb, bk_sb, kT),
                                              (wq_sb, bq_sb, qT)]:
                        pss = [psQK.tile([128, 512], F32, tag="qk", name="qk")
                               for _ in range(4)]
                        for j in range(4):
                            for tb in range(4):
                                nc.tensor.matmul(
                                    pss[tb][:],
                                    w_sb[j][:, :, ct * 128:(ct + 1) * 128],
                                    xb[j][:, :, tb * 512:(tb + 1) * 512],
                                    start=(j == 0), stop=(j == 3),
                                    perf_mode=DR)
                        for tb in range(4):
                            nc.vector.tensor_scalar(
                                out=dst[ct][:, tb * 512:(tb + 1) * 512],
                                in0=pss[tb][:],
                                scalar1=1.0 / (SX * SW),
                                scalar2=b_sb[:, ct:ct + 1],
                                op0=ALU.mult, op1=ALU.add)
                # v projection: stationary xb chunk, moving wv
                for tt in range(16):
                    ps = psV.tile([128, 512], F32, tag="v", name="v")
                    for j in range(4):
                        nc.tensor.matmul(
                            ps[:], xb[j][:, :, tt * 128:(tt + 1) * 128],
                            wv_sb[j][:], start=(j == 0), stop=(j == 3),
                            perf_mode=DR)
                    # store 16*v in fp8 (descale 1/512 * 16)
                    nc.vector.tensor_scalar_mul(
                        v_sb[tt // 2][:, tt % 2, :, 0:64],
                        ps[:].rearrange("p (h e) -> p h e", h=8),
                        16.0 / (SX * SW))
                    nc.vector.memset(v_sb[tt // 2][:, tt % 2, :, 64:65], 1.0)

        # prefetch FFN2 weights (resident) while attention runs
        for p in range(16):
            for ks in range(2):
                cc = 2 * p + ks
                nc.sync.dma_start(out=wout_sb[p][:, ks, :],
                                  in_=wout_d[cc * 128:(cc + 1) * 128, :])

        # ================= Phase C: attention =================
        for d in range(8):
            nc.sync.dma_start(out=xm[d][:], in_=xT_d[d * 128:(d + 1) * 128, :])
        with tc.tile_pool(name="psLo", bufs=2, space="PSUM") as psLop, \
             tc.tile_pool(name="psPV", bufs=2, space="PSUM") as psPVp, \
             tc.tile_pool(name="PT", bufs=2) as PTp, \
             tc.tile_pool(name="nrm", bufs=4) as nrmp:
            with nc.named_scope("phC_attn"):
                def normalize(hp, tb, pvs_e, pvs_o):
                    he, ho = 2 * hp, 2 * hp + 1
                    for hh, pvs in ((he, pvs_e), (ho, pvs_o)):
                        dbr = nrmp.tile([1, 512], BF, tag="dbr", name="dbr")
                        nc.scalar.copy(dbr[:], pvs[64:65, :])
                        bcp = psLp.tile([128, 512], F32, tag="Le",
                                        name="bcp")
                        nc.tensor.matmul(bcp[0:64, :], ones_c64[:],
                                         dbr[:], start=True, stop=True)
                        rec = nrmp.tile([64, 512], F32, tag="rec", name="rec")
                        nc.vector.reciprocal_approx_fast(
                            out=rec[:], in_=bcp[0:64, :])
                        row = (hh // 2)
                        half = (hh % 2) * 64
                        ydst = ymine if tb < 2 else ysent
                        ytsl = slice((tb % 2) * 512, (tb % 2) * 512 + 512)
                        yt = ydst[row][half:half + 64, ytsl]
                        nc.vector.tensor_tensor(
                            yt, pvs[0:64, :], rec[:], ALU.mult)
                        if tb >= 2:
                            nc.sync.dma_start(
                                out=cc_in[tb - 2][hh * 64:(hh + 1) * 64, :],
                                in_=yt)

                def emit_cc(tk):
                    nc.gpsimd.collective_compute(
                        "AllReduce", mybir.AluOpType.add,
                        ins=[cc_in[tk][:].opt()],
                        outs=[cc_out[tk][:].opt()],
                        replica_groups=[[0, 1], [2, 3], [4, 5], [6, 7]],
                    )

                # partner-token halves (tb=2 then 3) first, one head pair at
                # a time, so each token-half exchange chunk starts while
                # attention continues on the remaining halves
                ORDER = [(hp, tb) for tb in (2, 3, 0, 1) for hp in range(4)]
                A8 = 11.5415603  # 2^3/ln2 (e4m3 bit-space Schraudolph)
                B8 = 79.6
                LN8 = 2.0794415416798357

                # 18 ACT / 14 DVE chunks per (hp, tb): DVE also runs the
                # softmax normalize, so ACT takes a bit more of the exp
                EXP_PAT = [0, 1] * 16
                EXP_PAT[15] = 0
                EXP_PAT[31] = 0

                def exp_chunk(dst_e4, src_ps, idx):
                    if EXP_PAT[idx % 32] == 0:
                        nc.scalar.activation(dst_e4, src_ps, AF.Exp,
                                             scale=1.0 / 8.0, bias=ln8_128[:])
                    else:
                        nc.vector.tensor_scalar(
                            out=dst_e4.bitcast(mybir.dt.int8), in0=src_ps,
                            scalar1=A8 / 8.0, scalar2=B8,
                            op0=ALU.mult, op1=ALU.add)

                pending = None  # (hp, tb, pvs_e, pvs_o)
                cnt = 0
                for (hp, tb) in ORDER:
                    he, ho = 2 * hp, 2 * hp + 1
                    tsl = slice(tb * 512, (tb + 1) * 512)
                    pvs_e = psPVp.tile([65, 512], F32, tag="pve", name="pve")
                    pvs_o = psPVp.tile([65, 512], F32, tag="pvo", name="pvo")
                    prev = None  # (PT_e, PT_o, sp)
                    for sp in range(8):
                        PT_e = PTp.tile([128, 2, 512], E4, tag="pte",
                                        name="pte")
                        PT_o = PTp.tile([128, 2, 512], mybir.dt.int8,
                                        tag="pto", name="pto")
                        for ks in range(2):
                            s = 2 * sp + ks
                            ssl = slice(s * 128, (s + 1) * 128)
                            psL_e = psLp.tile([128, 512], F32, tag="Le",
                                              name="Le")
                            psL_o = psLop.tile([128, 512], F32, tag="Lo",
                                               name="Lo")
                            nc.tensor.matmul(
                                psL_e[:], kT[hp][0:64, ssl],
                                qT[hp][0:64, tsl],
                                start=True, stop=True, tile_position=(0, 0))
                            nc.tensor.matmul(
                                psL_o[:], kT[hp][64:128, ssl],
                                qT[hp][64:128, tsl],
                                start=True, stop=True, tile_position=(64, 0))
                            # drain previous s-pair's PVs while exp(s) runs
                            if ks == 0 and prev is not None:
                                PT_pe, PT_po, ps_ = prev
                                nc.tensor.matmul(
                                    pvs_e[:], v_sb[ps_][:, :, he, 0:65],
                                    PT_pe[:],
                                    start=(ps_ == 0), stop=(ps_ == 7),
                                    perf_mode=DR)
                                nc.tensor.matmul(
                                    pvs_o[:], v_sb[ps_][:, :, ho, 0:65],
                                    PT_po[:].bitcast(E4),
                                    start=(ps_ == 0), stop=(ps_ == 7),
                                    perf_mode=DR)
                            exp_chunk(PT_e[:, ks, :], psL_e[:], cnt)
                            exp_chunk(PT_o[:, ks, :].bitcast(E4), psL_o[:],
                                      cnt + 1)
                            cnt += 2
                        prev = (PT_e, PT_o, sp)
                    PT_pe, PT_po, ps_ = prev
                    nc.tensor.matmul(
                        pvs_e[:], v_sb[ps_][:, :, he, 0:65],
                        PT_pe[:], start=False, stop=True, perf_mode=DR)
                    nc.tensor.matmul(
                        pvs_o[:], v_sb[ps_][:, :, ho, 0:65],
                        PT_po[:].bitcast(E4),
                        start=False, stop=True, perf_mode=DR)
                    # deferred normalize of the previous iteration; its PE
                    # ops land behind this iteration's dense MM block so the
                    # PE never stalls on the ACT denominator copy
                    if pending is not None:
                        normalize(*pending)
                        # after all 4 head pairs of a partner-token half
                        if pending[1] >= 2 and pending[0] == 3:
                            emit_cc(pending[1] - 2)
                    pending = (hp, tb, pvs_e, pvs_o)
                normalize(*pending)
        pQKV.release()

        # ============ Phase D: exchange + residual + LN1 ============
        def ln_stats_apply(src, srcb, dst, dstb, lnw, lnb, lnws, lnbs,
                           psp, rowp, sqp, tmpp, tbb, fold16,
                           dst_narrow=False, seq_stats=False):
            """One 512-token block of transposed layernorm: stats via bf16
            ones-matmul on srcb, apply to src -> dst (f32) and optionally
            dstb (fp8, 16x-scaled via lnws/lnbs).  dst_narrow: dst tiles
            are [128, 512] (one block) rather than [128, TLOC].
            seq_stats: run sum then sum-of-squares through a single PSUM
            bank (for phases where only one bank is free)."""
            sl = slice(tbb * 512, (tbb + 1) * 512)
            dsl = slice(0, 512) if dst_narrow else sl
            if seq_stats:
                sqs = []
                psum_s = psp.tile([128, 512], F32, tag="Le", name="lnst")
                for d in range(8):
                    sq = sqp.tile([128, 512], BF, tag="sq", name="sq")
                    (nc.vector if d < 6 else nc.gpsimd).tensor_tensor(
                        sq[:], srcb[d][:, sl], srcb[d][:, sl], ALU.mult)
                    sqs.append(sq)
                    nc.tensor.matmul(psum_s[:], ones_mat[:], srcb[d][:, sl],
                                     start=(d == 0), stop=(d == 7))
                mean = rowp.tile([128, 512], F32, tag="mean", name="mean")
                nc.vector.tensor_scalar_mul(mean[:], psum_s[:], 1.0 / D)
                psum_q = psp.tile([128, 512], F32, tag="Le", name="lnst")
                for d in range(8):
                    nc.tensor.matmul(psum_q[:], ones_mat[:], sqs[d][:],
                                     start=(d == 0), stop=(d == 7))
            else:
                psum_s = psp.tile([128, 512], F32, tag="lns", name="lns")
                psum_q = psp.tile([128, 512], F32, tag="lnq", name="lnq")
                for d in range(8):
                    sq = sqp.tile([128, 512], BF, tag="sq", name="sq")
                    (nc.vector if d < 4 else nc.gpsimd).tensor_tensor(
                        sq[:], srcb[d][:, sl], srcb[d][:, sl], ALU.mult)
                    nc.tensor.matmul(psum_s[:], ones_mat[:], srcb[d][:, sl],
                                     start=(d == 0), stop=(d == 7))
                    nc.tensor.matmul(psum_q[:], ones_mat[:], sq[:],
                                     start=(d == 0), stop=(d == 7))
                mean = rowp.tile([128, 512], F32, tag="mean", name="mean")
                nc.vector.tensor_scalar_mul(mean[:], psum_s[:], 1.0 / D)
            m2 = rowp.tile([128, 512], F32, tag="m2", name="m2")
            nc.vector.tensor_tensor(m2[:], mean[:], mean[:], ALU.mult)
            var = m2  # in-place: var = psum_q/D - m2
            nc.vector.scalar_tensor_tensor(
                out=var[:], in0=psum_q[:], scalar=1.0 / D, in1=m2[:],
                op0=ALU.mult, op1=ALU.subtract)
            std = rowp.tile([128, 512], F32, tag="std", name="std")
            nc.scalar.activation(std[:], var[:], AF.Sqrt, bias=eps128[:])
            rstd = std  # in-place reciprocal
            nc.vector.reciprocal_approx_fast(out=rstd[:], in_=std[:])
            ms = rowp.tile([128, 512], F32, tag="ms", name="ms")
            nc.vector.tensor_tensor(ms[:], mean[:], rstd[:], ALU.mult)
            for d in range(8):
                # SBUF-only apply ops: offload the tail d-chunks to GPSIMD
                eng = nc.vector if d < 6 else nc.gpsimd
                tmp = tmpp.tile([128, 512], F32, tag="lt", name="lt")
                eng.tensor_tensor(tmp[:], src[d][:, sl], rstd[:],
                                  ALU.mult)
                tmp2 = tmp
                eng.tensor_tensor(tmp2[:], tmp[:], ms[:],
                                  ALU.subtract)
                nc.vector.tensor_scalar(
                    out=dst[d][:, dsl], in0=tmp2[:],
                    scalar1=lnw[:, d:d + 1], scalar2=lnb[:, d:d + 1],
                    op0=ALU.mult, op1=ALU.add)
                if dstb is not None:
                    # fp8 FFN1 input: 16*normalized (ln1 w/b folded into
                    # w_ff/b_ff on the host)
                    nc.vector.tensor_scalar_mul(
                        dstb[d // 2][:, d % 2, sl], tmp2[:], 16.0)

        with nc.named_scope("phD_exch_ln1"):
            # per token half: exchange arrives -> residual -> LN1, so
            # half 0 runs while attention still computes my-token halves
            for tbb in range(2):
                tsl = slice(tbb * 512, (tbb + 1) * 512)
                yp = [ypartp.tile([128, 512], BF, tag=f"yp{i}",
                                  name=f"yp{i}") for i in range(4)]
                for r4 in range(4):
                    nc.sync.dma_start(
                        out=yp[r4][:],
                        in_=cc_out[tbb][r4 * 128:(r4 + 1) * 128, :])
                    (nc.vector if r4 < 3 else nc.gpsimd).tensor_tensor(
                        yp[r4][:], yp[r4][:], ysent[r4][:, tsl],
                        ALU.subtract)
                # residual (rotated D order: chunks 0-3 mine, 4-7 partner);
                # v-bias is folded into xT on the host
                for d in range(8):
                    ysrc = (ymine[d][:, tsl] if d < 4
                            else yp[d - 4][:])
                    eng = nc.vector if d < 6 else nc.gpsimd
                    eng.tensor_tensor(
                        xsb[d][:, tsl], ysrc, xm[d][:, tsl], ALU.add)
                    eng.tensor_tensor(
                        xm[d][:, tsl], ysrc, xm[d][:, tsl], ALU.add)
                ln_stats_apply(xm, xsb, xm, xnb, lnw1_sb, lnb1_sb,
                               None, None,
                               psLp, lnrow, lnsq, lntmp, tbb, True,
                               seq_stats=True)
        pY.release()
        psLp.release()

        # ================= Phase E: FFN (fp8 DR) =================
        xn = xm      # LN1 output (f32, + b_out folded) in place
        r2 = xm      # FFN residual written back in place
        with tc.tile_pool(name="wff", bufs=2) as wffp, \
             tc.tile_pool(name="hbuf", bufs=1) as hbufp:
            with nc.named_scope("phE_ffn1"), \
                 tc.tile_pool(name="psH", bufs=4, space="PSUM") as psH:
                h_sb = [hbufp.tile([128, 2, TLOC], E4, tag=f"hb{p}",
                                   name=f"hb{p}") for p in range(16)]
                for blk in range(8):
                    wt = []
                    for j in range(4):
                        w = wffp.tile([128, 2, 512], E4, tag=f"wf{j}",
                                      name=f"wf{j}")
                        for ks in range(2):
                            dd = 2 * j + ks
                            nc.sync.dma_start(
                                out=w[:, ks, :],
                                in_=wff_d[dd * 128:(dd + 1) * 128,
                                          blk * 512:(blk + 1) * 512])
                        wt.append(w)
                    for j2 in range(4):
                        dt_i = blk * 4 + j2
                        for t2 in range(2):
                            sl = slice(t2 * 512, (t2 + 1) * 512)
                            ps = psH.tile([128, 512], F32, tag="h", name="h")
                            for j in range(4):
                                nc.tensor.matmul(
                                    ps[:],
                                    wt[j][:, :, j2 * 128:(j2 + 1) * 128],
                                    xnb[j][:, :, sl],
                                    start=(j == 0), stop=(j == 3),
                                    perf_mode=DR)
                            # h8 = relu(ps/64 + 8*bff) = 8*relu(ps/512+bff)
                            nc.scalar.activation(
                                h_sb[dt_i // 2][:, dt_i % 2, sl], ps[:],
                                AF.Relu,
                                bias=bff_sb[:, dt_i:dt_i + 1],
                                scale=SH / (SX1 * SWF))
            # FFN2 per token half so LN2(half0) overlaps FFN2(half1)
            with tc.tile_pool(name="psO", bufs=3, space="PSUM") as psO, \
                 tc.tile_pool(name="psD2", bufs=2, space="PSUM") as psD2, \
                 tc.tile_pool(name="lnrow2", bufs=1) as lnrow2, \
                 tc.tile_pool(name="lnsq2", bufs=3) as lnsq2, \
                 tc.tile_pool(name="lntmp2", bufs=2) as lntmp2, \
                 tc.tile_pool(name="ost", bufs=1) as ostp:
                with nc.named_scope("phE_ffn2"):
                    for tg in range(2):
                        sl = slice(tg * 512, (tg + 1) * 512)
                        for dd in range(8):
                            pso = psO.tile([128, 512], F32, tag="o", name="o")
                            for p in range(16):
                                nc.tensor.matmul(
                                    pso[:],
                                    wout_sb[p][:, :, dd * 128:(dd + 1) * 128],
                                    h_sb[p][:, :, sl],
                                    start=(p == 0), stop=(p == 15),
                                    perf_mode=DR)
                            # residual: xn already holds ln1out + b_out
                            nc.vector.scalar_tensor_tensor(
                                out=xsb[dd][:, sl], in0=pso[:],
                                scalar=1.0 / (SH * SWO),
                                in1=xn[dd][:, sl], op0=ALU.mult, op1=ALU.add)
                            nc.vector.scalar_tensor_tensor(
                                out=r2[dd][:, sl], in0=pso[:],
                                scalar=1.0 / (SH * SWO),
                                in1=xn[dd][:, sl], op0=ALU.mult, op1=ALU.add)
                        # LN2 + store for this token half
                        o32 = [ostp.tile([128, 512], F32, tag=f"o{d}",
                                         name=f"o{d}") for d in range(8)]
                        ln_stats_apply(
                            r2, xsb, o32,
                            None, lnw2_sb, lnb2_sb, None, None,
                            psD2, lnrow2, lnsq2, lntmp2, tg, False,
                            dst_narrow=True)
                        for d in range(8):
                            nc.sync.dma_start(
                                out=out_d[d * 128:(d + 1) * 128, sl],
                                in_=o32[d][:])
        lntmp.release()
        lnsq.release()
        lnrow.release()
        ypartp.release()
        pXNB.release()
        pWO.release()
        pXSB.release()
        pXM.release()
        dramp.release()
        constp.release()

    nc.compile()
    return nc


def _get_program():
    global _PROGRAM
    if _PROGRAM is None:
        _PROGRAM = _build_program()
    return _PROGRAM


def _rotations(hg):
    d0 = hg * 512
    drot = (np.arange(D) + d0) % D
    return d0, drot


def _make_in_maps(x, w_qkv, b_qkv, w_ff, b_ff, w_out, b_out,
                  ln1_w, ln1_b, ln2_w, ln2_b):
    # reference packs qkv interleaved: col(h, dh, sel) = h*192 + dh*3 + sel
    hd = np.arange(H * DH)
    qcols = (hd // DH) * (3 * DH) + (hd % DH) * 3
    kcols = qcols + 1
    vcols = qcols + 2
    in_maps = []
    # ln1 folded into the FFN1 weights/bias (FFN1 input is 16*normalized)
    bff_eff = 8.0 * (b_ff + ln1_b @ w_ff)
    for c in range(N_CORES):
        b = c // 2
        hg = c % 2
        t0 = hg * TLOC
        d0, drot = _rotations(hg)
        x_rot = np.concatenate([x[b, t0:t0 + TLOC, :],
                                x[b, TLOC - t0:T - t0, :]], axis=0)[:, drot]
        xT = np.ascontiguousarray(x_rot.T)          # [D, T]
        bv = b_qkv[vcols][drot]
        im = {
            # v-bias pre-folded into the residual input
            "xT": np.ascontiguousarray(xT[:, :TLOC] + bv[:, None]),
            "xb": np.ascontiguousarray((xT * SX).astype(E4NP)),
            "wq": np.ascontiguousarray(
                (w_qkv[drot][:, qcols[d0:d0 + 512]] * SW).astype(E4NP)),
            "wk": np.ascontiguousarray(
                (w_qkv[drot][:, kcols[d0:d0 + 512]] * SW).astype(E4NP)),
            "wv": np.ascontiguousarray(
                (w_qkv[drot][:, vcols[d0:d0 + 512]] * SW).astype(E4NP)),
            "bq": np.ascontiguousarray(
                b_qkv[qcols[d0:d0 + 512]].reshape(4, 128).T),
            "bk": np.ascontiguousarray(
                b_qkv[kcols[d0:d0 + 512]].reshape(4, 128).T),
            "wff": np.ascontiguousarray(
                (w_ff[drot, :] * ln1_w[drot][:, None] * SWF).astype(E4NP)),
            "bff": np.ascontiguousarray(bff_eff.reshape(32, 128).T),
            "wout": np.ascontiguousarray((w_out[:, drot] * SWO).astype(E4NP)),
            "lnw1": np.ascontiguousarray(ln1_w[drot].reshape(8, 128).T),
            "lnb1": np.ascontiguousarray(
                (ln1_b[drot] + b_out[drot]).reshape(8, 128).T),
            "lnw2": np.ascontiguousarray(ln2_w[drot].reshape(8, 128).T),
            "lnb2": np.ascontiguousarray(ln2_b[drot].reshape(8, 128).T),
        }
        in_maps.append(im)
    return in_maps


def _assemble(results):
    out = np.empty((B, T, D), dtype=np.float32)
    for c in range(N_CORES):
        b = c // 2
        hg = c % 2
        _, drot = _rotations(hg)
        inv = np.argsort(drot)
        out[b, hg * TLOC:(hg + 1) * TLOC, :] = results[c]["outT"].T[:, inv]
    return out


def _numpy_fallback(x, mask, w_qkv, b_qkv, w_ff, b_ff, w_out, b_out,
                    ln1_w, ln1_b, ln2_w, ln2_b):
    def ln(v, w, b):
        mu = v.mean(-1, keepdims=True)
        var = ((v - mu) ** 2).mean(-1, keepdims=True)
        return (v - mu) / np.sqrt(var + LN_EPS) * w + b
    b, t, _ = x.shape
    qkv = x @ w_qkv + b_qkv
    qkv = qkv.reshape(b, t, H, DH, 3).transpose(4, 0, 2, 1, 3)
    q, k, v = qkv[0], qkv[1], qkv[2]
    logits = np.einsum("bhtd,bhsd->bhts", q, k) / np.sqrt(DH)
    logits = logits + (1.0 - mask) * -10000.0
    m = logits.max(-1, keepdims=True)
    e = np.exp(logits - m)
    w = e / e.sum(-1, keepdims=True)
    y = np.einsum("bhts,bhsd->bhtd", w, v)
    y = y.transpose(0, 2, 1, 3).reshape(b, t, H * DH)
    x1 = ln(x + y, ln1_w, ln1_b)
    y2 = np.maximum(x1 @ w_ff + b_ff, 0.0) @ w_out + b_out
    return ln(x1 + y2, ln2_w, ln2_b).astype(np.float32)


def kernel(x, mask, w_qkv, b_qkv, w_ff, b_ff, w_out, b_out,
           ln1_w, ln1_b, ln2_w, ln2_b):
    args = [np.ascontiguousarray(np.asarray(a, dtype=np.float32))
            for a in (x, mask, w_qkv, b_qkv, w_ff, b_ff, w_out, b_out,
                      ln1_w, ln1_b, ln2_w, ln2_b)]
    (x, mask, w_qkv, b_qkv, w_ff, b_ff, w_out, b_out,
     ln1_w, ln1_b, ln2_w, ln2_b) = args

    if not np.all(mask == 1.0):
        return _numpy_fallback(x, mask, w_qkv, b_qkv, w_ff, b_ff, w_out, b_out,
                               ln1_w, ln1_b, ln2_w, ln2_b)

    _install_ntff_hook()
    from concourse.bass_utils import run_bass_kernel_spmd

    nc = _get_program()
    in_maps = _make_in_maps(x, w_qkv, b_qkv, w_ff, b_ff, w_out, b_out,
                            ln1_w, ln1_b, ln2_w, ln2_b)

    kw = {}
    if os.environ.get("BASSK_TRACE"):
        kw = dict(trace=True, trace_cores=[0],
                  tmpdir=os.environ.get("BASSK_TRACEDIR", "/tmp/kernel_trace"))
    res = run_bass_kernel_spmd(nc, in_maps, core_ids=list(range(N_CORES)), **kw)
    kernel._last_results = res
    return _assemble(res.results)


# revision 57
# speedup vs baseline: 1.3666x; 1.1242x over previous
"""Trainium2 Bass kernel for a dense transformer decoder layer (fp32 I/O).

Model: B=4, T=2048, H=16 heads, DH=64, D=1024, DFF=4096.
  qkv = x @ w_qkv + b_qkv ; non-causal attention (mask==1) ; residual+LN1 ;
  relu FFN (D->DFF->D) ; residual+LN2.

Sharding over 8 NeuronCores: core c handles batch b=c//2 and head-group
hg=c%2 (8 of 16 heads) for QKV+attention over the full sequence; the core
pair (2b, 2b+1) exchanges attention-output halves with a pairwise
AllReduce(add), and each core runs LN1/FFN/LN2 for its 1024-token half.

v3 design notes (vs v2):
  - QKV / FFN1 / FFN2 matmuls run fp8e4 DoubleRow (2 contraction rows per
    PE cell, ~1.4x tensor throughput).  Weights are host-pre-scaled into
    the e4m3 normal range (w_qkv*32, w_ff*32, w_out*64) and activations
    are scaled (x*16, ln1-out*16, h*8); the descale folds into the
    existing bias/residual ops so op count is unchanged.
  - Attention (hp, tb) loop runs partner-token halves (tb=2,3) first and
    emits the pairwise AllReduce in 4 per-head-pair chunks, overlapping
    the collective under the remaining attention compute.
  - FFN2 iterates token halves with w_out resident in SBUF (fp8), so
    LN2+store for half 0 overlaps FFN2 compute for half 1.
  - FFN1 folds relu+bias+descale+fp8-quant into one ACT op
    (relu(psum/64 + 8*bff) == 8*relu(psum/512 + bff)).
  - b_out folds into the LN1 bias of the fp32 path (xn is only consumed
    by the FFN2 residual add).
"""
import os
import sys
import types

import numpy as np
import ml_dtypes

if "/opt/trn_rl_repo" not in sys.path:
    sys.path.insert(0, "/opt/trn_rl_repo")

BF16NP = ml_dtypes.bfloat16
E4NP = ml_dtypes.float8_e4m3

B, T, H, DH = 4, 2048, 16, 64
D = H * DH            # 1024
DFF = 4096
LN_EPS = 1e-5
N_CORES = 8
TLOC = T // 2         # tokens per core in the FFN phase
HLOC = H // 2         # heads per core

A16 = 184.6650        # 2^7 / ln 2 (bf16 bit-space Schraudolph)
B16 = 16250.0

SX = 16.0             # x scale into QKV (fp8)
SW = 32.0             # w_qkv scale
SX1 = 16.0            # LN1-out scale into FFN1
SWF = 32.0            # w_ff scale
SH = 8.0              # h scale into FFN2
SWO = 64.0            # w_out scale

_PROGRAM = None


def _install_ntff_hook():
    try:
        import antenv
        if "antenv.axon_hooks" in sys.modules:
            return
        mod = types.ModuleType("antenv.axon_hooks")
        holder = [None]
        mod.set_axon_ntff_profile_hook = lambda h: holder.__setitem__(0, h)
        mod.get_axon_ntff_profile_hook = lambda: holder[0]
        sys.modules["antenv.axon_hooks"] = mod
        antenv.axon_hooks = mod
        from trn_agent_boot.trn_boot import _ntff_profile_via_ctypes
        mod.set_axon_ntff_profile_hook(
            _ntff_profile_via_ctypes("/opt/axon/libaxon_pjrt.so"))
    except Exception:
        pass


def _build_program():
    import concourse.bass as bass
    import concourse.mybir as mybir
    import concourse.tile as tile
    from concourse import bacc

    F32 = mybir.dt.float32
    BF = mybir.dt.bfloat16
    E4 = mybir.dt.float8e4
    I16 = mybir.dt.int16
    AF = mybir.ActivationFunctionType
    ALU = mybir.AluOpType
    DR = mybir.MatmulPerfMode.DoubleRow

    nc = bacc.Bacc("TRN2", target_bir_lowering=False, debug=False,
                   num_devices=N_CORES)

    xT_d = nc.dram_tensor("xT", [D, TLOC], F32, kind="ExternalInput").ap()
    xb_d = nc.dram_tensor("xb", [D, T], E4, kind="ExternalInput").ap()
    wq_d = nc.dram_tensor("wq", [D, 512], E4, kind="ExternalInput").ap()
    wk_d = nc.dram_tensor("wk", [D, 512], E4, kind="ExternalInput").ap()
    wv_d = nc.dram_tensor("wv", [D, 512], E4, kind="ExternalInput").ap()
    bq_d = nc.dram_tensor("bq", [128, 4], F32, kind="ExternalInput").ap()
    bk_d = nc.dram_tensor("bk", [128, 4], F32, kind="ExternalInput").ap()
    wff_d = nc.dram_tensor("wff", [D, DFF], E4, kind="ExternalInput").ap()
    bff_d = nc.dram_tensor("bff", [128, 32], F32, kind="ExternalInput").ap()
    wout_d = nc.dram_tensor("wout", [DFF, D], E4, kind="ExternalInput").ap()
    lnw1_d = nc.dram_tensor("lnw1", [128, 8], F32, kind="ExternalInput").ap()
    lnb1_d = nc.dram_tensor("lnb1", [128, 8], F32, kind="ExternalInput").ap()
    lnw2_d = nc.dram_tensor("lnw2", [128, 8], F32, kind="ExternalInput").ap()
    lnb2_d = nc.dram_tensor("lnb2", [128, 8], F32, kind="ExternalInput").ap()
    out_d = nc.dram_tensor("outT", [D, TLOC], F32, kind="ExternalOutput").ap()

    with tile.TileContext(nc) as tc:
        constp = tc.alloc_tile_pool(name="const", bufs=1)
        dramp = tc.alloc_tile_pool(name="dram", bufs=1, space="DRAM")

        eps128 = constp.tile([128, 1], F32)
        nc.vector.memset(eps128[:], LN_EPS)
        ln8_128 = constp.tile([128, 1], F32)
        nc.vector.memset(ln8_128[:], 2.0794415416798357)  # ln(8)
        ones_mat = constp.tile([128, 128], BF)
        nc.vector.memset(ones_mat[:].bitcast(mybir.dt.uint16), 0x3F80)
        ones_c64 = constp.tile([1, 64], BF)
        # 16.0: folds the 1/16 V-scale into the denominator broadcast
        nc.vector.memset(ones_c64[:].bitcast(mybir.dt.uint16), 0x4180)

        bias_tiles = {}
        for name, d_ap, w in [("bq", bq_d, 4), ("bk", bk_d, 4),
                              ("bff", bff_d, 32),
                              ("lnw1", lnw1_d, 8), ("lnb1", lnb1_d, 8),
                              ("lnw2", lnw2_d, 8), ("lnb2", lnb2_d, 8)]:
            t = constp.tile([128, w], F32, tag=name)
            nc.sync.dma_start(out=t[:], in_=d_ap)
            bias_tiles[name] = t
        bq_sb, bk_sb = bias_tiles["bq"], bias_tiles["bk"]
        bff_sb = bias_tiles["bff"]
        lnw1_sb, lnb1_sb = bias_tiles["lnw1"], bias_tiles["lnb1"]
        lnw2_sb, lnb2_sb = bias_tiles["lnw2"], bias_tiles["lnb2"]

        # ============ persistent SBUF state ============
        pXM = tc.alloc_tile_pool(name="pXM", bufs=1)
        xm = [pXM.tile([128, TLOC], F32, tag=f"xm{d}", name=f"xm{d}")
              for d in range(8)]
        pXSB = tc.alloc_tile_pool(name="pXSB", bufs=1)
        xsb = [pXSB.tile([128, TLOC], BF, tag=f"xsb{d}", name=f"xsb{d}")
               for d in range(8)]
        # resident fp8 FFN2 weights: pair p covers dff rows (2p,2p+1)*128
        pWO = tc.alloc_tile_pool(name="pWO", bufs=1)
        wout_sb = [pWO.tile([128, 2, D], E4, tag=f"wo{p}", name=f"wo{p}")
                   for p in range(16)]
        pXNB = tc.alloc_tile_pool(name="pXNB", bufs=1)
        xnb = [pXNB.tile([128, 2, TLOC], E4, tag=f"xnb{j}", name=f"xnb{j}")
               for j in range(4)]
        # phase-D pools allocated below the attention pools so LN1 work can
        # overlap late attention without SBUF-reuse anti-dependencies
        ypartp = tc.alloc_tile_pool(name="ypart", bufs=1)
        lnrow = tc.alloc_tile_pool(name="lnrow", bufs=1)
        lnsq = tc.alloc_tile_pool(name="lnsq", bufs=8)
        lntmp = tc.alloc_tile_pool(name="lntmp", bufs=2)
        # 2 PSUM banks shared by attention logits-even/bcast and LN1 stats
        psLp = tc.alloc_tile_pool(name="psLe", bufs=2, space="PSUM")
        pY = tc.alloc_tile_pool(name="pY", bufs=1)
        ymine = [pY.tile([128, TLOC], BF, tag=f"ym{i}", name=f"ym{i}")
                 for i in range(4)]
        ysent = [pY.tile([128, TLOC], BF, tag=f"ys{i}", name=f"ys{i}")
                 for i in range(4)]
        pQKV = tc.alloc_tile_pool(name="pQKV", bufs=1)
        qT = [pQKV.tile([128, T], BF, tag=f"qT{i}", name=f"qT{i}")
              for i in range(4)]
        kT = [pQKV.tile([128, T], BF, tag=f"kT{i}", name=f"kT{i}")
              for i in range(4)]
        # fp8 V for DoubleRow PV: s-pair m holds s-chunks (2m, 2m+1);
        # per head 64 dims + ones col (denominator row) + pad to 80
        v_sb = [pQKV.tile([128, 2, 8, 80], E4, tag=f"v{i}", name=f"v{i}")
                for i in range(8)]
        # collective chunks split by partner-token half (bf16 payload)
        cc_in = [dramp.tile([512, 512], BF, tag=f"ccin{i}", name=f"ccin{i}")
                 for i in range(2)]
        cc_out = [dramp.tile([512, 512], BF, tag=f"ccout{i}", name=f"ccout{i}")
                  for i in range(2)]

        # ================= Phase B: QKV projections (fp8 DR) ============
        with tc.tile_pool(name="xbp", bufs=1) as xbp, \
             tc.tile_pool(name="wqk", bufs=1) as wqkp, \
             tc.tile_pool(name="psQK", bufs=4, space="PSUM") as psQK, \
             tc.tile_pool(name="psV", bufs=2, space="PSUM") as psV:
            xb = [xbp.tile([128, 2, T], E4, tag=f"xb{j}", name=f"xb{j}")
                  for j in range(4)]
            wq_sb = [wqkp.tile([128, 2, 512], E4, tag=f"wq{j}", name=f"wq{j}")
                     for j in range(4)]
            wk_sb = [wqkp.tile([128, 2, 512], E4, tag=f"wk{j}", name=f"wk{j}")
                     for j in range(4)]
            wv_sb = [wqkp.tile([128, 2, 512], E4, tag=f"wv{j}", name=f"wv{j}")
                     for j in range(4)]
            for j in range(4):
                for ks in range(2):
                    dd = 2 * j + ks
                    rows = slice(dd * 128, (dd + 1) * 128)
                    nc.sync.dma_start(out=xb[j][:, ks, :], in_=xb_d[rows, :])
                    nc.sync.dma_start(out=wk_sb[j][:, ks, :], in_=wk_d[rows, :])
                    nc.sync.dma_start(out=wq_sb[j][:, ks, :], in_=wq_d[rows, :])
                    nc.sync.dma_start(out=wv_sb[j][:, ks, :], in_=wv_d[rows, :])
            with nc.named_scope("phB_qkv"):
                # q/k projections: stationary w chunk, moving xb
                for ct in range(4):
                    for (w_sb, b_sb, dst) in [(wk_sb, bk_sb, kT),
                                              (wq_sb, bq_sb, qT)]:
                        pss = [psQK.tile([128, 512], F32, tag="qk", name="qk")
                               for _ in range(4)]
                        for j in range(4):
                            for tb in range(4):
                                nc.tensor.matmul(
                                    pss[tb][:],
                                    w_sb[j][:, :, ct * 128:(ct + 1) * 128],
                                    xb[j][:, :, tb * 512:(tb + 1) * 512],
                                    start=(j == 0), stop=(j == 3),
                                    perf_mode=DR)
                        for tb in range(4):
                            nc.vector.tensor_scalar(
                                out=dst[ct][:, tb * 512:(tb + 1) * 512],
                                in0=pss[tb][:],
                                scalar1=1.0 / (SX * SW),
                                scalar2=b_sb[:, ct:ct + 1],
                                op0=ALU.mult, op1=ALU.add)
                # v projection: stationary xb chunk, moving wv
                for tt in range(16):
                    ps = psV.tile([128, 512], F32, tag="v", name="v")
                    for j in range(4):
                        nc.tensor.matmul(
                            ps[:], xb[j][:, :, tt * 128:(tt + 1) * 128],
                            wv_sb[j][:], start=(j == 0), stop=(j == 3),
                            perf_mode=DR)
                    # store 16*v in fp8 (descale 1/512 * 16)
                    nc.vector.tensor_scalar_mul(
                        v_sb[tt // 2][:, tt % 2, :, 0:64],
                        ps[:].rearrange("p (h e) -> p h e", h=8),
                        16.0 / (SX * SW))
                    nc.vector.memset(v_sb[tt // 2][:, tt % 2, :, 64:65], 1.0)

        # prefetch FFN2 weights (resident) while attention runs
        for p in range(16):
            for ks in range(2):
                cc = 2 * p + ks
                nc.sync.dma_start(out=wout_sb[p][:, ks, :],
                                  in_=wout_d[cc * 128:(cc + 1) * 128, :])

        # ================= Phase C: attention =================
        for d in range(8):
            nc.sync.dma_start(out=xm[d][:], in_=xT_d[d * 128:(d + 1) * 128, :])
        with tc.tile_pool(name="psLo", bufs=2, space="PSUM") as psLop, \
             tc.tile_pool(name="psPV", bufs=2, space="PSUM") as psPVp, \
             tc.tile_pool(name="PT", bufs=3) as PTp, \
             tc.tile_pool(name="nrm", bufs=4) as nrmp:
            with nc.named_scope("phC_attn"):
                def normalize(hp, tb, pvs_e, pvs_o):
                    he, ho = 2 * hp, 2 * hp + 1
                    for hh, pvs in ((he, pvs_e), (ho, pvs_o)):
                        dbr = nrmp.tile([1, 512], BF, tag="dbr", name="dbr")
                        nc.scalar.copy(dbr[:], pvs[64:65, :])
                        bcp = psLp.tile([128, 512], F32, tag="Le",
                                        name="bcp")
                        nc.tensor.matmul(bcp[0:64, :], ones_c64[:],
                                         dbr[:], start=True, stop=True)
                        rec = nrmp.tile([64, 512], F32, tag="rec", name="rec")
                        nc.vector.reciprocal_approx_fast(
                            out=rec[:], in_=bcp[0:64, :])
                        row = (hh // 2)
                        half = (hh % 2) * 64
                        ydst = ymine if tb < 2 else ysent
                        ytsl = slice((tb % 2) * 512, (tb % 2) * 512 + 512)
                        yt = ydst[row][half:half + 64, ytsl]
                        nc.vector.tensor_tensor(
                            yt, pvs[0:64, :], rec[:], ALU.mult)
                        if tb >= 2:
                            nc.sync.dma_start(
                                out=cc_in[tb - 2][hh * 64:(hh + 1) * 64, :],
                                in_=yt)

                def emit_cc(tk):
                    nc.gpsimd.collective_compute(
                        "AllReduce", mybir.AluOpType.add,
                        ins=[cc_in[tk][:].opt()],
                        outs=[cc_out[tk][:].opt()],
                        replica_groups=[[0, 1], [2, 3], [4, 5], [6, 7]],
                    )

                # partner-token halves (tb=2 then 3) first, one head pair at
                # a time, so each token-half exchange chunk starts while
                # attention continues on the remaining halves
                ORDER = [(hp, tb) for tb in (2, 3, 0, 1) for hp in range(4)]
                A8 = 11.5415603  # 2^3/ln2 (e4m3 bit-space Schraudolph)
                B8 = 79.6
                LN8 = 2.0794415416798357

                # 18 ACT / 14 DVE chunks per (hp, tb): DVE also runs the
                # softmax normalize, so ACT takes a bit more of the exp
                EXP_PAT = [0, 1] * 16
                EXP_PAT[15] = 0
                EXP_PAT[31] = 0

                def exp_chunk(dst_e4, src_ps, idx):
                    if EXP_PAT[idx % 32] == 0:
                        nc.scalar.activation(dst_e4, src_ps, AF.Exp,
                                             scale=1.0 / 8.0, bias=ln8_128[:])
                    else:
                        nc.vector.tensor_scalar(
                            out=dst_e4.bitcast(mybir.dt.int8), in0=src_ps,
                            scalar1=A8 / 8.0, scalar2=B8,
                            op0=ALU.mult, op1=ALU.add)

                pending = None  # (hp, tb, pvs_e, pvs_o)
                cnt = 0
                for (hp, tb) in ORDER:
                    he, ho = 2 * hp, 2 * hp + 1
                    tsl = slice(tb * 512, (tb + 1) * 512)
                    pvs_e = psPVp.tile([65, 512], F32, tag="pve", name="pve")
                    pvs_o = psPVp.tile([65, 512], F32, tag="pvo", name="pvo")

                    def pv_mm(entry, last):
                        PT_pe, PT_po, ps_ = entry
                        nc.tensor.matmul(
                            pvs_e[:], v_sb[ps_][:, :, he, 0:65],
                            PT_pe[:], start=(ps_ == 0),
                            stop=(ps_ == 7) and last, perf_mode=DR)
                        nc.tensor.matmul(
                            pvs_o[:], v_sb[ps_][:, :, ho, 0:65],
                            PT_po[:].bitcast(E4), start=(ps_ == 0),
                            stop=(ps_ == 7) and last, perf_mode=DR)

                    pipe = []  # 2-deep PV pipeline: exp gets ~2 sp of slack
                    for sp in range(8):
                        PT_e = PTp.tile([128, 2, 512], E4, tag="pte",
                                        name="pte")
                        PT_o = PTp.tile([128, 2, 512], mybir.dt.int8,
                                        tag="pto", name="pto")
                        for ks in range(2):
                            s = 2 * sp + ks
                            ssl = slice(s * 128, (s + 1) * 128)
                            psL_e = psLp.tile([128, 512], F32, tag="Le",
                                              name="Le")
                            psL_o = psLop.tile([128, 512], F32, tag="Lo",
                                               name="Lo")
                            nc.tensor.matmul(
                                psL_e[:], kT[hp][0:64, ssl],
                                qT[hp][0:64, tsl],
                                start=True, stop=True, tile_position=(0, 0))
                            nc.tensor.matmul(
                                psL_o[:], kT[hp][64:128, ssl],
                                qT[hp][64:128, tsl],
                                start=True, stop=True, tile_position=(64, 0))
                            # drain a queued s-pair's PVs while exp(s) runs
                            if ks == 0 and len(pipe) >= 2:
                                pv_mm(pipe.pop(0), True)
                            exp_chunk(PT_e[:, ks, :], psL_e[:], cnt)
                            exp_chunk(PT_o[:, ks, :].bitcast(E4), psL_o[:],
                                      cnt + 1)
                            cnt += 2
                        pipe.append((PT_e, PT_o, sp))
                    for entry in pipe:
                        pv_mm(entry, True)
                    # deferred normalize of the previous iteration; its PE
                    # ops land behind this iteration's dense MM block so the
                    # PE never stalls on the ACT denominator copy
                    if pending is not None:
                        normalize(*pending)
                        # after all 4 head pairs of a partner-token half
                        if pending[1] >= 2 and pending[0] == 3:
                            emit_cc(pending[1] - 2)
                    pending = (hp, tb, pvs_e, pvs_o)
                normalize(*pending)
        pQKV.release()

        # ============ Phase D: exchange + residual + LN1 ============
        def ln_stats_apply(src, srcb, dst, dstb, lnw, lnb, lnws, lnbs,
                           psp, rowp, sqp, tmpp, tbb, fold16,
                           dst_narrow=False, seq_stats=False):
            """One 512-token block of transposed layernorm: stats via bf16
            ones-matmul on srcb, apply to src -> dst (f32) and optionally
            dstb (fp8, 16x-scaled via lnws/lnbs).  dst_narrow: dst tiles
            are [128, 512] (one block) rather than [128, TLOC].
            seq_stats: run sum then sum-of-squares through a single PSUM
            bank (for phases where only one bank is free)."""
            sl = slice(tbb * 512, (tbb + 1) * 512)
            dsl = slice(0, 512) if dst_narrow else sl
            if seq_stats:
                sqs = []
                psum_s = psp.tile([128, 512], F32, tag="Le", name="lnst")
                for d in range(8):
                    sq = sqp.tile([128, 512], BF, tag="sq", name="sq")
                    (nc.vector if d < 6 else nc.gpsimd).tensor_tensor(
                        sq[:], srcb[d][:, sl], srcb[d][:, sl], ALU.mult)
                    sqs.append(sq)
                    nc.tensor.matmul(psum_s[:], ones_mat[:], srcb[d][:, sl],
                                     start=(d == 0), stop=(d == 7))
                mean = rowp.tile([128, 512], F32, tag="mean", name="mean")
                nc.vector.tensor_scalar_mul(mean[:], psum_s[:], 1.0 / D)
                psum_q = psp.tile([128, 512], F32, tag="Le", name="lnst")
                for d in range(8):
                    nc.tensor.matmul(psum_q[:], ones_mat[:], sqs[d][:],
                                     start=(d == 0), stop=(d == 7))
            else:
                psum_s = psp.tile([128, 512], F32, tag="lns", name="lns")
                psum_q = psp.tile([128, 512], F32, tag="lnq", name="lnq")
                for d in range(8):
                    sq = sqp.tile([128, 512], BF, tag="sq", name="sq")
                    (nc.vector if d < 4 else nc.gpsimd).tensor_tensor(
                        sq[:], srcb[d][:, sl], srcb[d][:, sl], ALU.mult)
                    nc.tensor.matmul(psum_s[:], ones_mat[:], srcb[d][:, sl],
                                     start=(d == 0), stop=(d == 7))
                    nc.tensor.matmul(psum_q[:], ones_mat[:], sq[:],
                                     start=(d == 0), stop=(d == 7))
                mean = rowp.tile([128, 512], F32, tag="mean", name="mean")
                nc.vector.tensor_scalar_mul(mean[:], psum_s[:], 1.0 / D)
            m2 = rowp.tile([128, 512], F32, tag="m2", name="m2")
            nc.vector.tensor_tensor(m2[:], mean[:], mean[:], ALU.mult)
            var = m2  # in-place: var = psum_q/D - m2
            nc.vector.scalar_tensor_tensor(
                out=var[:], in0=psum_q[:], scalar=1.0 / D, in1=m2[:],
                op0=ALU.mult, op1=ALU.subtract)
            std = rowp.tile([128, 512], F32, tag="std", name="std")
            nc.scalar.activation(std[:], var[:], AF.Sqrt, bias=eps128[:])
            rstd = std  # in-place reciprocal
            nc.vector.reciprocal_approx_fast(out=rstd[:], in_=std[:])
            ms = rowp.tile([128, 512], F32, tag="ms", name="ms")
            nc.vector.tensor_tensor(ms[:], mean[:], rstd[:], ALU.mult)
            for d in range(8):
                # SBUF-only apply ops: offload the tail d-chunks to GPSIMD
                eng = nc.vector if d < 6 else nc.gpsimd
                tmp = tmpp.tile([128, 512], F32, tag="lt", name="lt")
                eng.tensor_tensor(tmp[:], src[d][:, sl], rstd[:],
                                  ALU.mult)
                tmp2 = tmp
                eng.tensor_tensor(tmp2[:], tmp[:], ms[:],
                                  ALU.subtract)
                nc.vector.tensor_scalar(
                    out=dst[d][:, dsl], in0=tmp2[:],
                    scalar1=lnw[:, d:d + 1], scalar2=lnb[:, d:d + 1],
                    op0=ALU.mult, op1=ALU.add)
                if dstb is not None:
                    # fp8 FFN1 input: 16*normalized (ln1 w/b folded into
                    # w_ff/b_ff on the host)
                    nc.vector.tensor_scalar_mul(
                        dstb[d // 2][:, d % 2, sl], tmp2[:], 16.0)

        with nc.named_scope("phD_exch_ln1"):
            # per token half: exchange arrives -> residual -> LN1, so
            # half 0 runs while attention still computes my-token halves
            for tbb in range(2):
                tsl = slice(tbb * 512, (tbb + 1) * 512)
                yp = [ypartp.tile([128, 512], BF, tag=f"yp{i}",
                                  name=f"yp{i}") for i in range(4)]
                for r4 in range(4):
                    nc.sync.dma_start(
                        out=yp[r4][:],
                        in_=cc_out[tbb][r4 * 128:(r4 + 1) * 128, :])
                    (nc.vector if r4 < 3 else nc.gpsimd).tensor_tensor(
                        yp[r4][:], yp[r4][:], ysent[r4][:, tsl],
                        ALU.subtract)
                # residual (rotated D order: chunks 0-3 mine, 4-7 partner);
                # v-bias is folded into xT on the host
                for d in range(8):
                    ysrc = (ymine[d][:, tsl] if d < 4
                            else yp[d - 4][:])
                    eng = nc.vector if d < 6 else nc.gpsimd
                    eng.tensor_tensor(
                        xsb[d][:, tsl], ysrc, xm[d][:, tsl], ALU.add)
                    eng.tensor_tensor(
                        xm[d][:, tsl], ysrc, xm[d][:, tsl], ALU.add)
                ln_stats_apply(xm, xsb, xm, xnb, lnw1_sb, lnb1_sb,
                               None, None,
                               psLp, lnrow, lnsq, lntmp, tbb, True,
                               seq_stats=True)
        pY.release()
        psLp.release()

        # ================= Phase E: FFN (fp8 DR) =================
        xn = xm      # LN1 output (f32, + b_out folded) in place
        r2 = xm      # FFN residual written back in place
        with tc.tile_pool(name="wff", bufs=2) as wffp, \
             tc.tile_pool(name="hbuf", bufs=1) as hbufp:
            with nc.named_scope("phE_ffn1"), \
                 tc.tile_pool(name="psH", bufs=4, space="PSUM") as psH:
                h_sb = [hbufp.tile([128, 2, TLOC], E4, tag=f"hb{p}",
                                   name=f"hb{p}") for p in range(16)]
                for blk in range(8):
                    wt = []
                    for j in range(4):
                        w = wffp.tile([128, 2, 512], E4, tag=f"wf{j}",
                                      name=f"wf{j}")
                        for ks in range(2):
                            dd = 2 * j + ks
                            nc.sync.dma_start(
                                out=w[:, ks, :],
                                in_=wff_d[dd * 128:(dd + 1) * 128,
                                          blk * 512:(blk + 1) * 512])
                        wt.append(w)
                    for j2 in range(4):
                        dt_i = blk * 4 + j2
                        for t2 in range(2):
                            sl = slice(t2 * 512, (t2 + 1) * 512)
                            ps = psH.tile([128, 512], F32, tag="h", name="h")
                            for j in range(4):
                                nc.tensor.matmul(
                                    ps[:],
                                    wt[j][:, :, j2 * 128:(j2 + 1) * 128],
                                    xnb[j][:, :, sl],
                                    start=(j == 0), stop=(j == 3),
                                    perf_mode=DR)
                            # h8 = relu(ps/64 + 8*bff) = 8*relu(ps/512+bff)
                            nc.scalar.activation(
                                h_sb[dt_i // 2][:, dt_i % 2, sl], ps[:],
                                AF.Relu,
                                bias=bff_sb[:, dt_i:dt_i + 1],
                                scale=SH / (SX1 * SWF))
            # FFN2 per token half so LN2(half0) overlaps FFN2(half1)
            with tc.tile_pool(name="psO", bufs=3, space="PSUM") as psO, \
                 tc.tile_pool(name="psD2", bufs=2, space="PSUM") as psD2, \
                 tc.tile_pool(name="lnrow2", bufs=1) as lnrow2, \
                 tc.tile_pool(name="lnsq2", bufs=3) as lnsq2, \
                 tc.tile_pool(name="lntmp2", bufs=2) as lntmp2, \
                 tc.tile_pool(name="ost", bufs=1) as ostp:
                with nc.named_scope("phE_ffn2"):
                    for tg in range(2):
                        sl = slice(tg * 512, (tg + 1) * 512)
                        for dd in range(8):
                            pso = psO.tile([128, 512], F32, tag="o", name="o")
                            for p in range(16):
                                nc.tensor.matmul(
                                    pso[:],
                                    wout_sb[p][:, :, dd * 128:(dd + 1) * 128],
                                    h_sb[p][:, :, sl],
                                    start=(p == 0), stop=(p == 15),
                                    perf_mode=DR)
                            # residual: xn already holds ln1out + b_out
                            nc.vector.scalar_tensor_tensor(
                                out=xsb[dd][:, sl], in0=pso[:],
                                scalar=1.0 / (SH * SWO),
                                in1=xn[dd][:, sl], op0=ALU.mult, op1=ALU.add)
                            nc.vector.scalar_tensor_tensor(
                                out=r2[dd][:, sl], in0=pso[:],
                                scalar=1.0 / (SH * SWO),
                                in1=xn[dd][:, sl], op0=ALU.mult, op1=ALU.add)
                        # LN2 + store for this token half
                        o32 = [ostp.tile([128, 512], F32, tag=f"o{d}",
                                         name=f"o{d}") for d in range(8)]
                        ln_stats_apply(
                            r2, xsb, o32,
                            None, lnw2_sb, lnb2_sb, None, None,
                            psD2, lnrow2, lnsq2, lntmp2, tg, False,
                            dst_narrow=True)
                        for d in range(8):
                            nc.sync.dma_start(
                                out=out_d[d * 128:(d + 1) * 128, sl],
                                in_=o32[d][:])
        lntmp.release()
        lnsq.release()
        lnrow.release()
        ypartp.release()
        pXNB.release()
        pWO.release()
        pXSB.release()
        pXM.release()
        dramp.release()
        constp.release()

    nc.compile()
    return nc


def _get_program():
    global _PROGRAM
    if _PROGRAM is None:
        _PROGRAM = _build_program()
    return _PROGRAM


def _rotations(hg):
    d0 = hg * 512
    drot = (np.arange(D) + d0) % D
    return d0, drot


def _make_in_maps(x, w_qkv, b_qkv, w_ff, b_ff, w_out, b_out,
                  ln1_w, ln1_b, ln2_w, ln2_b):
    # reference packs qkv interleaved: col(h, dh, sel) = h*192 + dh*3 + sel
    hd = np.arange(H * DH)
    qcols = (hd // DH) * (3 * DH) + (hd % DH) * 3
    kcols = qcols + 1
    vcols = qcols + 2
    in_maps = []
    # ln1 folded into the FFN1 weights/bias (FFN1 input is 16*normalized)
    bff_eff = 8.0 * (b_ff + ln1_b @ w_ff)
    for c in range(N_CORES):
        b = c // 2
        hg = c % 2
        t0 = hg * TLOC
        d0, drot = _rotations(hg)
        x_rot = np.concatenate([x[b, t0:t0 + TLOC, :],
                                x[b, TLOC - t0:T - t0, :]], axis=0)[:, drot]
        xT = np.ascontiguousarray(x_rot.T)          # [D, T]
        bv = b_qkv[vcols][drot]
        im = {
            # v-bias pre-folded into the residual input
            "xT": np.ascontiguousarray(xT[:, :TLOC] + bv[:, None]),
            "xb": np.ascontiguousarray((xT * SX).astype(E4NP)),
            "wq": np.ascontiguousarray(
                (w_qkv[drot][:, qcols[d0:d0 + 512]] * SW).astype(E4NP)),
            "wk": np.ascontiguousarray(
                (w_qkv[drot][:, kcols[d0:d0 + 512]] * SW).astype(E4NP)),
            "wv": np.ascontiguousarray(
                (w_qkv[drot][:, vcols[d0:d0 + 512]] * SW).astype(E4NP)),
            "bq": np.ascontiguousarray(
                b_qkv[qcols[d0:d0 + 512]].reshape(4, 128).T),
            "bk": np.ascontiguousarray(
                b_qkv[kcols[d0:d0 + 512]].reshape(4, 128).T),
            "wff": np.ascontiguousarray(
                (w_ff[drot, :] * ln1_w[drot][:, None] * SWF).astype(E4NP)),
            "bff": np.ascontiguousarray(bff_eff.reshape(32, 128).T),
            "wout": np.ascontiguousarray((w_out[:, drot] * SWO).astype(E4NP)),
            "lnw1": np.ascontiguousarray(ln1_w[drot].reshape(8, 128).T),
            "lnb1": np.ascontiguousarray(
                (ln1_b[drot] + b_out[drot]).reshape(8, 128).T),
            "lnw2": np.ascontiguousarray(ln2_w[drot].reshape(8, 128).T),
            "lnb2": np.ascontiguousarray(ln2_b[drot].reshape(8, 128).T),
        }
        in_maps.append(im)
    return in_maps


def _assemble(results):
    out = np.empty((B, T, D), dtype=np.float32)
    for c in range(N_CORES):
        b = c // 2
        hg = c % 2
        _, drot = _rotations(hg)
        inv = np.argsort(drot)
        out[b, hg * TLOC:(hg + 1) * TLOC, :] = results[c]["outT"].T[:, inv]
    return out


def _numpy_fallback(x, mask, w_qkv, b_qkv, w_ff, b_ff, w_out, b_out,
                    ln1_w, ln1_b, ln2_w, ln2_b):
    def ln(v, w, b):
        mu = v.mean(-1, keepdims=True)
        var = ((v - mu) ** 2).mean(-1, keepdims=True)
        return (v - mu) / np.sqrt(var + LN_EPS) * w + b
    b, t, _ = x.shape
    qkv = x @ w_qkv + b_qkv
    qkv = qkv.reshape(b, t, H, DH, 3).transpose(4, 0, 2, 1, 3)
    q, k, v = qkv[0], qkv[1], qkv[2]
    logits = np.einsum("bhtd,bhsd->bhts", q, k) / np.sqrt(DH)
    logits = logits + (1.0 - mask) * -10000.0
    m = logits.max(-1, keepdims=True)
    e = np.exp(logits - m)
    w = e / e.sum(-1, keepdims=True)
    y = np.einsum("bhts,bhsd->bhtd", w, v)
    y = y.transpose(0, 2, 1, 3).reshape(b, t, H * DH)
    x1 = ln(x + y, ln1_w, ln1_b)
    y2 = np.maximum(x1 @ w_ff + b_ff, 0.0) @ w_out + b_out
    return ln(x1 + y2, ln2_w, ln2_b).astype(np.float32)


def kernel(x, mask, w_qkv, b_qkv, w_ff, b_ff, w_out, b_out,
           ln1_w, ln1_b, ln2_w, ln2_b):
    args = [np.ascontiguousarray(np.asarray(a, dtype=np.float32))
            for a in (x, mask, w_qkv, b_qkv, w_ff, b_ff, w_out, b_out,
                      ln1_w, ln1_b, ln2_w, ln2_b)]
    (x, mask, w_qkv, b_qkv, w_ff, b_ff, w_out, b_out,
     ln1_w, ln1_b, ln2_w, ln2_b) = args

    if not np.all(mask == 1.0):
        return _numpy_fallback(x, mask, w_qkv, b_qkv, w_ff, b_ff, w_out, b_out,
                               ln1_w, ln1_b, ln2_w, ln2_b)

    _install_ntff_hook()
    from concourse.bass_utils import run_bass_kernel_spmd

    nc = _get_program()
    in_maps = _make_in_maps(x, w_qkv, b_qkv, w_ff, b_ff, w_out, b_out,
                            ln1_w, ln1_b, ln2_w, ln2_b)

    kw = {}
    if os.environ.get("BASSK_TRACE"):
        kw = dict(trace=True, trace_cores=[0],
                  tmpdir=os.environ.get("BASSK_TRACEDIR", "/tmp/kernel_trace"))
    res = run_bass_kernel_spmd(nc, in_maps, core_ids=list(range(N_CORES)), **kw)
    kernel._last_results = res
    return _assemble(res.results)
